# revision 1
# baseline (speedup 1.0000x reference)
"""Trainium2 Bass kernel for the DependencyAnalyzer GNN problem.

Computation (reference semantics):
    h = relu(features @ W_node + b_node)                  # [N, H]
    2x: agg = scatter_add(h[src] -> dst);  h = relu((h + agg) @ W_conv + b_conv)
    out = stack([ (m*h) @ (m*h).T,  h @ h.T ])            # m = (nodes == 2)

Strategy (8 NeuronCores, SPMD):
  - Host reformats the edge list into per-core dense adjacency blocks
    A'^T [src=8192, dst_local=1024] in bf16, with the identity folded in
    (A' = A + S_c) so that A' @ h == h_block + agg_block.
  - Every core computes h0 for all nodes (cheap, replicated); round
    matmuls use bf16 hi/lo splits packed side by side in the stationary
    operand for fp32-grade accuracy at bf16 speed.
  - One 256KB AllGather per round exchanges the per-core h blocks.
  - similarity/function_deps are single float32r (tf32-like) matmuls per
    output tile; the function_deps mask is applied to the own-row operand
    and, between the two output passes, in place to the shared rhs.
  - Each core writes its 1024-row slice of both 8192x8192 outputs (64MB).
"""

import numpy as np
import ml_dtypes

import concourse.bass as bass
import concourse.mybir as mybir
import concourse.tile as tile
from concourse import masks
from concourse.bass_utils import run_bass_kernel_spmd

N = 8192          # nodes
NB = 1024         # nodes per core block
NCORES = 8
F = 10            # feature dim
FA = F + 1        # +1 ones row (bias fold)
H = 64            # hidden dim
KT = N // 128     # 64 src k-tiles
MT = NB // 128    # 8 own m-tiles
NT = N // 512     # 16 n-tiles of 512
F32 = mybir.dt.float32
F32R = mybir.dt.float32r
BF16 = mybir.dt.bfloat16
RELU = mybir.ActivationFunctionType.Relu

LAST_RESULT = None  # BassKernelResults of the most recent run (for test harness)


def _ensure_trace_hook():
    """Best-effort: register the NTFF profiling hook for trace=True runs.

    The agent image's ``antenv`` package lacks ``axon_hooks``; recreate it
    in-process and install the ctypes-based hook from trn_agent_boot so
    ``run_bass_kernel_spmd(trace=True)`` can capture HW exec times.
    Silently no-ops if anything is missing — plain runs are unaffected.
    """
    import sys as _sys
    import types as _types

    try:
        if "antenv.axon_hooks" in _sys.modules:
            return
        import antenv as _antenv

        mod = _types.ModuleType("antenv.axon_hooks")
        _state = {"hook": None}
        mod.set_axon_ntff_profile_hook = lambda h: _state.__setitem__("hook", h)
        mod.get_axon_ntff_profile_hook = lambda: _state["hook"]
        _sys.modules["antenv.axon_hooks"] = mod
        _antenv.axon_hooks = mod

        from trn_agent_boot.trn_boot import _ntff_profile_via_ctypes

        so_path = "/opt/axon/libaxon_pjrt.so"
        import os as _os

        if _os.path.exists(so_path):
            hook = _ntff_profile_via_ctypes(so_path)
            if hook is not None:
                mod.set_axon_ntff_profile_hook(hook)
    except Exception:
        pass


def _legalize_waits(nc, max_waits=1):
    """This walrus build accepts at most one sync-wait per lowered HW
    instruction; hoist extra waits onto standalone EventSemaphore
    instructions on the same (in-order) engine queue."""
    n_fixed = 0
    for f in nc.m.functions:
        for bb in f.blocks:
            new_list = []
            for ins in bb.instructions:
                si = ins.sync_info
                if si is not None and len(si.on_wait) > max_waits:
                    waits = list(si.on_wait)
                    for w in waits[: len(waits) - max_waits]:
                        ev = mybir.InstEventSemaphore(
                            name=f"{ins.name}-w-{w.ant_name}",
                            ins=[],
                            outs=[],
                            sync_info=mybir.SyncInfo(on_wait=[w], on_update=[]),
                            engine=ins.engine,
                        )
                        new_list.append(ev)
                    ins.sync_info = mybir.SyncInfo(
                        on_wait=waits[len(waits) - max_waits :],
                        on_update=list(si.on_update),
                    )
                    n_fixed += 1
                new_list.append(ins)
            bb.instructions = new_list
    return n_fixed


def _build_nc():
    nc = bass.Bass(num_devices=NCORES)

    # ---- external I/O (same program on all cores; per-core data differs) ----
    # featT3/W3: K-stacked bf16 hi/lo decomposition of [features.T; ones] and
    # [W_node; b_node] so one bf16 matmul computes the fp32-accurate product:
    # [f_hi; f_lo; f_hi] . [W_hi; W_hi; W_lo] = f.W + b - f_lo.W_lo
    featT = nc.declare_dram_parameter("featT3", [3 * FA, N], BF16, isOutput=False)
    WnA = nc.declare_dram_parameter("W3", [3 * FA, H], BF16, isOutput=False)
    Wc2h = nc.declare_dram_parameter("Wc2h", [2 * H, H], BF16, isOutput=False)
    Wc2l = nc.declare_dram_parameter("Wc2l", [2 * H, H], BF16, isOutput=False)
    bc = nc.declare_dram_parameter("bc", [H, 1], F32, isOutput=False)
    nodes_ownT = nc.declare_dram_parameter("nodes_ownT", [128, MT], F32, isOutput=False)
    nodes_all = nc.declare_dram_parameter("nodes_all", [1, N], BF16, isOutput=False)
    F8 = mybir.dt.float8e4
    AT = nc.declare_dram_parameter("AT", [N, NB], F8, isOutput=False)
    out_ext = nc.declare_dram_parameter("out", [2, NB, N], F32, isOutput=True)

    # ---- internal DRAM (collective bounce buffers) ----
    ag1a_in = nc.dram_tensor("ag1a_in", [NB // 2, 128], BF16)
    ag1a_out = nc.dram_tensor("ag1a_out", [N // 2, 128], BF16, addr_space="Shared")
    ag1b_in = nc.dram_tensor("ag1b_in", [NB // 2, 128], BF16)
    ag1b_out = nc.dram_tensor("ag1b_out", [N // 2, 128], BF16, addr_space="Shared")
    ag2_in = nc.dram_tensor("ag2_in", [H, NB], F32R)
    ag2_out = nc.dram_tensor("ag2_out", [NCORES * H, NB], F32R, addr_space="Shared")
    rg = [list(range(NCORES))]

    with tile.TileContext(nc, num_cores=NCORES) as tc:
        with tc.tile_pool(name="persist", bufs=1) as persist:
            # ---------------- constants / small inputs (issued first) -------
            wn_s = persist.tile([3 * FA, H], BF16)
            nc.sync.dma_start(out=wn_s[:], in_=WnA[:])
            wc2h_s = persist.tile([2 * H, H], BF16)
            nc.sync.dma_start(out=wc2h_s[:], in_=Wc2h[:])
            wc2l_s = persist.tile([2 * H, H], BF16)
            nc.sync.dma_start(out=wc2l_s[:], in_=Wc2l[:])
            bc_s = persist.tile([H, 1], F32)
            nc.sync.dma_start(out=bc_s[:], in_=bc[:])
            ident = persist.tile([128, 128], BF16)
            masks.make_identity(nc, ident[:])
            ones_s = persist.tile([1, 128], BF16)
            nc.vector.memset(ones_s[:], 1.0)
            dummy_s = persist.tile([1, 512], BF16)
            nc.vector.memset(dummy_s[:], 0.0)

            def absorb(pt, parts, free):
                # Dummy full-tile matmul: soaks up PSUM pool-boundary WAR
                # waits on PE so real matmuls stay within the ISA's sync
                # wait budget.
                nc.tensor.matmul(
                    pt[:, :],
                    dummy_s[0:1, 0:parts],
                    dummy_s[0:1, 0:free],
                    start=True,
                    stop=True,
                )

            # final-h operand for the big output matmuls (filled in round 2)
            hT_r = persist.tile([H, NB], F32R)      # own block, T layout, f32r

            with (
                tc.tile_pool(name="apool", bufs=KT // 2) as apool,
                tc.tile_pool(name="hilo", bufs=KT) as hilopool,
            ):
                # ------------- phase 1: h0 for all nodes (replicated) -------
                h0_tiles = []
                with (
                    tc.tile_pool(name="ph1", bufs=2) as ph1,
                    tc.tile_pool(name="pp1", bufs=4, space="PSUM") as pp1,
                ):
                    # features first so h0 overlaps the big A-load
                    ft_halves = []
                    for half in range(2):
                        ft_h = ph1.tile([3 * FA, N // 2], BF16, tag=f"ft{half}", bufs=1)
                        nc.sync.dma_start(
                            out=ft_h[:],
                            in_=featT[:, half * (N // 2) : (half + 1) * (N // 2)],
                        )
                        ft_halves.append(ft_h)

                    # adjacency blocks, resident in SBUF for both rounds
                    # (2 k-tiles per DMA: [256, NB] -> [128, 2*NB])
                    a2_tiles = []
                    for j in range(KT // 2):
                        at = apool.tile([128, 2 * NB], BF16, name=f"a{j}", tag="A")
                        src = AT[j * 256 : (j + 1) * 256, :].rearrange(
                            "(t p) n -> p t n", p=128
                        )
                        # fp8 in DRAM, cast to bf16 on the way in (SWDGE)
                        nc.gpsimd.dma_start(
                            out=at[:].rearrange("p (t n) -> p t n", t=2), in_=src
                        )
                        a2_tiles.append(at)

                    for k in range(KT):
                        ft_s = ft_halves[k // (KT // 2)]
                        kk = k % (KT // 2)
                        ps = pp1.tile([128, H], F32, tag="p64", bufs=4)
                        if k == 0:
                            absorb(ps, 128, H)
                        nc.tensor.matmul(
                            ps[:],
                            ft_s[:, kk * 128 : (kk + 1) * 128],
                            wn_s[:],
                            start=True,
                            stop=True,
                        )
                        h0f = ph1.tile([128, H], F32, tag="h0f", bufs=4)
                        nc.scalar.activation(h0f[:], ps[:], RELU)
                        hl = hilopool.tile([128, 128], BF16, name=f"h0hl{k}", tag="HL")
                        nc.vector.tensor_copy(hl[:, 0:H], h0f[:])
                        nc.vector.tensor_sub(hl[:, H:128], h0f[:], hl[:, 0:H])
                        h0_tiles.append(hl)

                # ------------- phase 2: two message-passing rounds ----------
                cur_tiles = h0_tiles
                rnd2_korder = list(range(KT))
                for rnd in (1, 2):
                    with (
                        tc.tile_pool(name=f"rd{rnd}", bufs=1) as rd,
                        tc.tile_pool(name=f"prd{rnd}", bufs=1, space="PSUM") as prd,
                    ):
                        # agg'T: psum rows 0:64 = (A'@hi)T, rows 64:128 =
                        # (A'@lo)T, then h_newT = relu(W_conv^T @ agg' + b)
                        # via bf16 hi/lo of agg against bf16 hi/lo of W_conv.
                        if rnd == 1:
                            hT32 = rd.tile([H, NB], F32, tag="hT32")
                        else:
                            hT32 = hT_r  # round to f32r for the output matmuls
                        for n in range(2):
                            psa = prd.tile([128, 512], F32, tag="psa", bufs=2)
                            if n == 0:
                                absorb(psa, 128, 512)
                            ks = range(KT) if rnd == 1 else rnd2_korder
                            for ki, k in enumerate(ks):
                                off = (k % 2) * NB + n * 512
                                nc.tensor.matmul(
                                    psa[:],
                                    cur_tiles[k],
                                    a2_tiles[k // 2][:, off : off + 512],
                                    start=(ki == 0),
                                    stop=(ki == KT - 1),
                                )
                            agg_hi = rd.tile([128, 512], BF16, tag="agghi", bufs=2)
                            nc.vector.tensor_copy(agg_hi[:], psa[:])
                            agg_h32 = rd.tile([128, 512], F32, tag="aggh32", bufs=2)
                            nc.vector.tensor_copy(agg_h32[:], agg_hi[:])
                            agg_lo = rd.tile([128, 512], BF16, tag="agglo", bufs=2)
                            nc.vector.tensor_sub(agg_lo[:], psa[:], agg_h32[:])
                            psw = prd.tile([H, 512], F32, tag="psw", bufs=2)
                            if n == 0:
                                absorb(psw, H, 512)
                            nc.tensor.matmul(
                                psw[:], wc2h_s[:], agg_hi[:], start=True, stop=False
                            )
                            nc.tensor.matmul(
                                psw[:], wc2h_s[:], agg_lo[:], start=False, stop=False
                            )
                            nc.tensor.matmul(
                                psw[:], wc2l_s[:], agg_hi[:], start=False, stop=True
                            )
                            nc.scalar.activation(
                                hT32[:, n * 512 : (n + 1) * 512],
                                psw[:],
                                RELU,
                                bias=bc_s[:],
                            )

                        if rnd == 1:
                            # split to bf16 hi/lo, transpose own block to
                            # normal layout, all-gather, unpack for round 2.
                            hiT = rd.tile([H, NB], BF16, tag="hiT")
                            nc.vector.tensor_copy(hiT[:], hT32[:])
                            hi32b = rd.tile([H, NB], F32, tag="hi32b")
                            nc.vector.tensor_copy(hi32b[:], hiT[:])
                            loT = rd.tile([H, NB], BF16, tag="loT")
                            nc.vector.tensor_sub(loT[:], hT32[:], hi32b[:])
                            # two half all-gathers: the second one's latency
                            # overlaps round 2's first batch of matmuls
                            for half, (agi, ago) in enumerate(
                                [(ag1a_in, ag1a_out), (ag1b_in, ag1b_out)]
                            ):
                                for mm in range(MT // 2):
                                    m = half * (MT // 2) + mm
                                    pst = prd.tile([128, 128], BF16, tag="pst", bufs=2)
                                    nc.tensor.transpose(
                                        pst[:, 0:H],
                                        hiT[:, m * 128 : (m + 1) * 128],
                                        ident[0:H, 0:H],
                                    )
                                    nc.tensor.transpose(
                                        pst[:, H:128],
                                        loT[:, m * 128 : (m + 1) * 128],
                                        ident[0:H, 0:H],
                                    )
                                    nrm = rd.tile([128, 128], BF16, tag="nrm", bufs=4)
                                    nc.vector.tensor_copy(nrm[:], pst[:])
                                    nc.sync.dma_start(
                                        out=agi[mm * 128 : (mm + 1) * 128, :],
                                        in_=nrm[:],
                                    )
                                nc.gpsimd.collective_compute(
                                    "AllGather",
                                    mybir.AluOpType.bypass,
                                    replica_groups=rg,
                                    ins=[agi[:]],
                                    outs=[ago[:]],
                                )
                            cur_tiles = [None] * KT
                            korder = []
                            for half, ago in [(0, ag1a_out), (1, ag1b_out)]:
                                for g in range(8):
                                    hl8 = hilopool.tile(
                                        [128, 4 * 128], BF16,
                                        name=f"h1hl{half}_{g}", tag="HL8", bufs=16,
                                    )
                                    src = ago[
                                        g * 512 : (g + 1) * 512, :
                                    ].rearrange("(t p) c -> p t c", p=128)
                                    nc.sync.dma_start(
                                        out=hl8[:].rearrange(
                                            "p (t c) -> p t c", t=4
                                        ),
                                        in_=src,
                                    )
                                    for t in range(4):
                                        k = g * 8 + half * 4 + t
                                        cur_tiles[k] = hl8[:, t * 128 : (t + 1) * 128]
                                        korder.append(k)
                            rnd2_korder = korder
                        else:
                            # final h (f32r): all-gather the T-layout block
                            # for the output matmuls.
                            nc.sync.dma_start(out=ag2_in[:], in_=hT_r[:])
                            nc.gpsimd.collective_compute(
                                "AllGather",
                                mybir.AluOpType.bypass,
                                replica_groups=rg,
                                ins=[ag2_in[:]],
                                outs=[ag2_out[:]],
                            )

            # ---------------- phase 3: sim / fdeps + output -----------------
            # (A/hilo pools released -> plenty of SBUF for f32r operands)
            # fdeps tile = sim psum * rowmask (per-partition scalar)
            #            * colmask (broadcast tensor): one fused DVE op,
            # so function_deps needs no matmuls of its own.
            with (
                tc.tile_pool(name="ph3", bufs=1) as ph3,
                tc.tile_pool(name="stg", bufs=3) as stg,
                tc.tile_pool(name="pp3", bufs=8, space="PSUM") as pp3,
            ):
                rhs_r = ph3.tile([H, N], F32R, tag="rhs")
                for c in range(NCORES):
                    nc.sync.dma_start(
                        out=rhs_r[:, c * NB : (c + 1) * NB],
                        in_=ag2_out[c * H : (c + 1) * H, :],
                    )
                mask_all = ph3.tile([1, N], BF16, tag="maskall")
                nc.sync.dma_start(out=mask_all[:], in_=nodes_all[:])
                nc.vector.tensor_single_scalar(
                    mask_all[:], mask_all[:], 2.0, mybir.AluOpType.is_equal
                )
                nodes_tp = ph3.tile([128, MT], F32, tag="nodestp")
                nc.sync.dma_start(out=nodes_tp[:], in_=nodes_ownT[:])
                maskT = ph3.tile([128, MT], F32, tag="maskT")
                nc.vector.tensor_single_scalar(
                    maskT[:], nodes_tp[:], 2.0, mybir.AluOpType.is_equal
                )
                # column mask broadcast to 128 partitions (K=1 matmuls), f32
                colmask = ph3.tile([128, N], F32, tag="colmask")
                for n in range(NT):
                    nsl = slice(n * 512, (n + 1) * 512)
                    psm = pp3.tile([128, 512], F32, tag="ps3", bufs=8)
                    nc.tensor.matmul(
                        psm[:], ones_s[:], mask_all[:, nsl], start=True, stop=True
                    )
                    nc.vector.tensor_copy(colmask[:, nsl], psm[:])

                first = True
                for m in range(MT):
                    msl = slice(m * 128, (m + 1) * 128)
                    rowm = maskT[:, m : m + 1]
                    for ng in range(4):
                        ngsl = slice(ng * 2048, (ng + 1) * 2048)
                        stA = stg.tile([128, 2048], F32, tag="stA", bufs=3)
                        stB = stg.tile([128, 2048], F32, tag="stB", bufs=3)
                        for j in range(4):
                            n = ng * 4 + j
                            nsl = slice(n * 512, (n + 1) * 512)
                            jsl = slice(j * 512, (j + 1) * 512)
                            ps3 = pp3.tile([128, 512], F32, tag="ps3", bufs=8)
                            if first:
                                absorb(ps3, 128, 512)
                                first = False
                            nc.tensor.matmul(
                                ps3[:],
                                hT_r[:, msl],
                                rhs_r[:, nsl],
                                start=True,
                                stop=True,
                            )
                            nc.scalar.copy(stA[:, jsl], ps3[:])
                            nc.vector.scalar_tensor_tensor(
                                stB[:, jsl],
                                ps3[:],
                                rowm,
                                colmask[:, nsl],
                                mybir.AluOpType.mult,
                                mybir.AluOpType.mult,
                            )
                        nc.sync.dma_start(out=out_ext[1, msl, ngsl], in_=stA[:])
                        nc.sync.dma_start(out=out_ext[0, msl, ngsl], in_=stB[:])
    _legalize_waits(nc)
    return nc


def _host_prep(features, W_node, b_node, W_conv, b_conv, nodes, edges):
    features = np.asarray(features, np.float32)
    W_node = np.asarray(W_node, np.float32)
    b_node = np.asarray(b_node, np.float32)
    W_conv = np.asarray(W_conv, np.float32)
    b_conv = np.asarray(b_conv, np.float32)
    nodes = np.asarray(nodes)
    edges = np.asarray(edges)

    def _hilo(x):
        hi = x.astype(ml_dtypes.bfloat16)
        lo = (x - hi.astype(np.float32)).astype(ml_dtypes.bfloat16)
        return hi, lo

    # [features.T; ones] and [W_node; b_node], K-stacked for bf16 hi/lo:
    # [fa_hi; fa_lo_z; fa_hi] . [Wa_hi; Wa_hi; Wa_lo] ~= f@W + b
    fa = np.concatenate([features.T, np.ones((1, N), np.float32)], axis=0)
    Wa = np.concatenate([W_node, b_node[None, :]], axis=0)
    fa_hi, fa_lo = _hilo(fa)
    fa_lo_z = fa_lo.copy()
    fa_lo_z[F, :] = 0  # no double-counted bias
    Wa_hi, Wa_lo = _hilo(Wa)
    featT3 = np.concatenate([fa_hi, fa_lo_z, fa_hi], axis=0)  # [33, N] bf16
    W3 = np.concatenate([Wa_hi, Wa_hi, Wa_lo], axis=0)  # [33, H] bf16

    Wc_hi, Wc_lo = _hilo(W_conv)
    Wc2h = np.concatenate([Wc_hi, Wc_hi], axis=0)  # [128, H] bf16
    Wc2l = np.concatenate([Wc_lo, Wc_lo], axis=0)
    bc = b_conv.reshape(H, 1)
    nodes_f = nodes.astype(np.float32).reshape(1, N)

    src = edges[:, 0].astype(np.int64)
    dst = edges[:, 1].astype(np.int64)
    in_maps = []
    for c in range(NCORES):
        sel = (dst >= c * NB) & (dst < (c + 1) * NB)
        idx = src[sel] * NB + (dst[sel] - c * NB)
        cnt = np.bincount(idx, minlength=N * NB).astype(np.float32).reshape(N, NB)
        cnt[c * NB + np.arange(NB), np.arange(NB)] += 1.0  # fold identity
        assert cnt.max() <= 16, "adjacency counts exceed exact fp8 range"
        in_maps.append(
            {
                "featT3": featT3,
                "W3": W3,
                "Wc2h": Wc2h,
                "Wc2l": Wc2l,
                "bc": bc,
                "nodes_ownT": np.ascontiguousarray(
                    nodes_f[0, c * NB : (c + 1) * NB].reshape(MT, 128).T
                ),
                "nodes_all": nodes_f.astype(ml_dtypes.bfloat16),
                "AT": cnt.astype(ml_dtypes.float8_e4m3),
            }
        )
    return in_maps


def kernel(features, W_node, b_node, W_conv, b_conv, nodes, edges, **kw):
    global LAST_RESULT
    _ensure_trace_hook()
    in_maps = _host_prep(features, W_node, b_node, W_conv, b_conv, nodes, edges)
    nc = _build_nc()
    res = run_bass_kernel_spmd(nc, in_maps, core_ids=list(range(NCORES)))
    LAST_RESULT = res
    out = np.empty((2, N, N), np.float32)
    for c in range(NCORES):
        out[:, c * NB : (c + 1) * NB, :] = res.results[c]["out"]
    return out


if __name__ == "__main__":
    np.random.seed(0)
    feats = np.random.randn(N, F).astype(np.float32)
    ins = {
        "features": feats,
        "W_node": (np.random.randn(F, H) * 0.1).astype(np.float32),
        "b_node": (np.random.randn(H) * 0.1).astype(np.float32),
        "W_conv": (np.random.randn(H, H) * 0.05).astype(np.float32),
        "b_conv": (np.random.randn(H) * 0.05).astype(np.float32),
        "nodes": np.random.randint(0, 5, N, dtype=np.int32),
        "edges": np.random.randint(0, N, (524288, 2), dtype=np.int32),
    }
    out = kernel(**ins)
    print(out.shape, out.dtype)



# revision 2
# speedup vs baseline: 1.3057x; 1.3057x over previous
"""Trainium2 Bass kernel for the DependencyAnalyzer GNN problem.

Computation (reference semantics):
    h = relu(features @ W_node + b_node)                  # [N, H]
    2x: agg = scatter_add(h[src] -> dst);  h = relu((h + agg) @ W_conv + b_conv)
    out = stack([ (m*h) @ (m*h).T,  h @ h.T ])            # m = (nodes == 2)

Strategy (8 NeuronCores, SPMD):
  - Host reformats the edge list into per-core dense adjacency blocks
    A'^T [src=8192, dst_local=1024] in fp8e4 (counts <= 16 are exact),
    with the identity folded in (A' = A + S_c) so A' @ h == h + agg.
  - h is carried as a 2-component fp8e4 decomposition (hi + lo ~ 8 mantissa
    bits); the A' matmuls run in fp8 DoubleRow perf mode: each instruction
    consumes TWO k-tiles (lhsT [128,2,128] h-comps, rhs [128,2,512] A rows)
    at half the per-column cost of bf16.
  - One small fp8 AllGather per round exchanges the per-core h blocks
    (round 1 in node-major layout for the round-2 stationary, round 2 in
    H-major layout for the output-phase moving operand). A tiny warmup
    collective at t=0 absorbs the CC-init barrier off the critical path.
  - similarity = (hi+lo)^T (hi+lo) via 2 DoubleRow matmuls per 512-col
    output tile (slot trick: [hi;lo]x[hi;hi] + [hi;0]x[lo;lo]); the
    function_deps tile is the same psum times row/col masks (one DVE op).
  - Outputs are staged as float16 scaled by 0.25 (|out|/4 < 65504), DMA'd
    as 32MB/core instead of 64MB, and rescaled to fp32 on the host.
"""

import numpy as np
import ml_dtypes

import concourse.bass as bass
import concourse.mybir as mybir
import concourse.tile as tile
from concourse import masks
from concourse.bass_utils import run_bass_kernel_spmd

N = 8192          # nodes
NB = 1024         # nodes per core block
NCORES = 8
F = 10            # feature dim
FA = F + 1        # +1 ones row (bias fold)
H = 64            # hidden dim
KT = N // 128     # 64 src k-tiles
NPAIR = KT // 2   # 32 k-tile pairs (DoubleRow)
MT = NB // 128    # 8 own m-tiles
NT = N // 512     # 16 n-tiles of 512
F32 = mybir.dt.float32
BF16 = mybir.dt.bfloat16
F16 = mybir.dt.float16
F8 = mybir.dt.float8e4
RELU = mybir.ActivationFunctionType.Relu
COPY = mybir.ActivationFunctionType.Copy
DR = mybir.MatmulPerfMode.DoubleRow

LAST_RESULT = None  # BassKernelResults of the most recent run (for test harness)


def _ensure_trace_hook():
    """Best-effort: register the NTFF profiling hook for trace=True runs.

    The agent image's ``antenv`` package lacks ``axon_hooks``; recreate it
    in-process and install the ctypes-based hook from trn_agent_boot so
    ``run_bass_kernel_spmd(trace=True)`` can capture HW exec times.
    Silently no-ops if anything is missing — plain runs are unaffected.
    """
    import sys as _sys
    import types as _types

    try:
        if "antenv.axon_hooks" in _sys.modules:
            return
        import antenv as _antenv

        mod = _types.ModuleType("antenv.axon_hooks")
        _state = {"hook": None}
        mod.set_axon_ntff_profile_hook = lambda h: _state.__setitem__("hook", h)
        mod.get_axon_ntff_profile_hook = lambda: _state["hook"]
        _sys.modules["antenv.axon_hooks"] = mod
        _antenv.axon_hooks = mod

        from trn_agent_boot.trn_boot import _ntff_profile_via_ctypes

        so_path = "/opt/axon/libaxon_pjrt.so"
        import os as _os

        if _os.path.exists(so_path):
            hook = _ntff_profile_via_ctypes(so_path)
            if hook is not None:
                mod.set_axon_ntff_profile_hook(hook)
    except Exception:
        pass


def _legalize_waits(nc, max_waits=1):
    """This walrus build accepts at most one sync-wait per lowered HW
    instruction; hoist extra waits onto standalone EventSemaphore
    instructions on the same (in-order) engine queue."""
    n_fixed = 0
    for f in nc.m.functions:
        for bb in f.blocks:
            new_list = []
            for ins in bb.instructions:
                si = ins.sync_info
                if si is not None and len(si.on_wait) > max_waits:
                    waits = list(si.on_wait)
                    for w in waits[: len(waits) - max_waits]:
                        ev = mybir.InstEventSemaphore(
                            name=f"{ins.name}-w-{w.ant_name}",
                            ins=[],
                            outs=[],
                            sync_info=mybir.SyncInfo(on_wait=[w], on_update=[]),
                            engine=ins.engine,
                        )
                        new_list.append(ev)
                    ins.sync_info = mybir.SyncInfo(
                        on_wait=waits[len(waits) - max_waits :],
                        on_update=list(si.on_update),
                    )
                    n_fixed += 1
                new_list.append(ins)
            bb.instructions = new_list
    return n_fixed


def _build_nc():
    nc = bass.Bass(num_devices=NCORES)

    # ---- external I/O (same program on all cores; per-core data differs) ----
    # featT3/W3: K-stacked bf16 hi/lo decomposition of [features.T; ones] and
    # [W_node; b_node] so one bf16 matmul computes the fp32-accurate product:
    # [f_hi; f_lo; f_hi] . [W_hi; W_hi; W_lo] = f.W + b - f_lo.W_lo
    featT = nc.declare_dram_parameter("featT3", [3 * FA, N], BF16, isOutput=False)
    WnA = nc.declare_dram_parameter("W3", [3 * FA, H], BF16, isOutput=False)
    Wc2h = nc.declare_dram_parameter("Wc2h", [2 * H, H], BF16, isOutput=False)
    Wc2l = nc.declare_dram_parameter("Wc2l", [2 * H, H], BF16, isOutput=False)
    bc = nc.declare_dram_parameter("bc", [H, 1], F32, isOutput=False)
    nodes_ownT = nc.declare_dram_parameter("nodes_ownT", [128, MT], F32, isOutput=False)
    nodes_all = nc.declare_dram_parameter("nodes_all", [1, N], BF16, isOutput=False)
    AT = nc.declare_dram_parameter("AT", [N, NB], F8, isOutput=False)
    out_ext = nc.declare_dram_parameter("out", [2, NB, N], F16, isOutput=True)

    # ---- internal DRAM (collective bounce buffers) ----
    warm_in = nc.dram_tensor("warm_in", [1, 128], F8)
    warm_out = nc.dram_tensor("warm_out", [NCORES, 128], F8, addr_space="Shared")
    ag1a_in = nc.dram_tensor("ag1a_in", [NB // 2, 128], F8)
    ag1a_out = nc.dram_tensor("ag1a_out", [N // 2, 128], F8, addr_space="Shared")
    ag1b_in = nc.dram_tensor("ag1b_in", [NB // 2, 128], F8)
    ag1b_out = nc.dram_tensor("ag1b_out", [N // 2, 128], F8, addr_space="Shared")
    ag2_in = nc.dram_tensor("ag2_in", [128, NB], F8)
    ag2_out = nc.dram_tensor("ag2_out", [NCORES * 128, NB], F8, addr_space="Shared")
    rg = [list(range(NCORES))]

    with tile.TileContext(nc, num_cores=NCORES) as tc:
        with tc.tile_pool(name="persist", bufs=1) as persist:
            # warmup collective: absorbs the one-time CC-init barrier while
            # the input DMAs and h0 run; payload contents are irrelevant.
            nc.gpsimd.collective_compute(
                "AllGather",
                mybir.AluOpType.bypass,
                replica_groups=rg,
                ins=[warm_in[:]],
                outs=[warm_out[:]],
            )

            # ---------------- constants / small inputs (issued first) -------
            wn_s = persist.tile([3 * FA, H], BF16)
            nc.sync.dma_start(out=wn_s[:], in_=WnA[:])
            wc2h_s = persist.tile([2 * H, H], BF16)
            nc.sync.dma_start(out=wc2h_s[:], in_=Wc2h[:])
            wc2l_s = persist.tile([2 * H, H], BF16)
            nc.sync.dma_start(out=wc2l_s[:], in_=Wc2l[:])
            bc_s = persist.tile([H, 1], F32)
            nc.sync.dma_start(out=bc_s[:], in_=bc[:])
            ident = persist.tile([128, 128], BF16)
            masks.make_identity(nc, ident[:])
            ones_s = persist.tile([1, 128], BF16)
            nc.vector.memset(ones_s[:], 1.0)
            dummy_s = persist.tile([1, 512], BF16)
            nc.vector.memset(dummy_s[:], 0.0)
            zero64 = persist.tile([H, 512], F32)
            nc.vector.memset(zero64[:], 0.0)

            def absorb(pt, parts, free):
                # Dummy full-tile matmul: soaks up PSUM pool-boundary WAR
                # waits on PE so real matmuls stay within the ISA's sync
                # wait budget.
                nc.tensor.matmul(
                    pt[:, :],
                    dummy_s[0:1, 0:parts],
                    dummy_s[0:1, 0:free],
                    start=True,
                    stop=True,
                )

            # final-h fp8 components (own block, T layout), for phase 3
            hi8T = persist.tile([H, NB], F8)
            lo8T = persist.tile([H, NB], F8)

            with (
                tc.tile_pool(name="apool", bufs=NPAIR) as apool,
                tc.tile_pool(name="hilo", bufs=NPAIR + 16) as hilopool,
            ):
                # ------------- phase 1: h0 for all nodes (replicated) -------
                pair_tiles = []   # round-1 lhsT pair tiles [128, 256] fp8
                with (
                    tc.tile_pool(name="ph1", bufs=2) as ph1,
                    tc.tile_pool(name="pp1", bufs=4, space="PSUM") as pp1,
                ):
                    # features first so h0 overlaps the big A-load
                    ft_halves = []
                    for half in range(2):
                        ft_h = ph1.tile([3 * FA, N // 2], BF16, tag=f"ft{half}", bufs=1)
                        nc.sync.dma_start(
                            out=ft_h[:],
                            in_=featT[:, half * (N // 2) : (half + 1) * (N // 2)],
                        )
                        ft_halves.append(ft_h)

                    # adjacency pair blocks, fp8 resident in SBUF for both
                    # rounds; col layout (n-half, k-slot, 512) so each
                    # DoubleRow rhs is a contiguous-slice rearrange.
                    a2_tiles = []
                    for j in range(NPAIR):
                        at = apool.tile([128, 2 * NB], F8, name=f"a{j}", tag="A")
                        for hnf in range(2):
                            nc.gpsimd.dma_start(
                                out=at[
                                    :, hnf * NB : (hnf + 1) * NB
                                ].rearrange("p (t n) -> p t n", t=2),
                                in_=AT[
                                    j * 256 : (j + 1) * 256,
                                    hnf * 512 : (hnf + 1) * 512,
                                ].rearrange("(t p) n -> p t n", p=128),
                            )
                        a2_tiles.append(at)

                    for k in range(KT):
                        ft_s = ft_halves[k // (KT // 2)]
                        kk = k % (KT // 2)
                        ps = pp1.tile([128, H], F32, tag="p64", bufs=4)
                        if k == 0:
                            absorb(ps, 128, H)
                        nc.tensor.matmul(
                            ps[:],
                            ft_s[:, kk * 128 : (kk + 1) * 128],
                            wn_s[:],
                            start=True,
                            stop=True,
                        )
                        if k % 2 == 0:
                            pr = hilopool.tile(
                                [128, 256], F8, name=f"h0p{k // 2}", tag="HP"
                            )
                            pair_tiles.append(pr)
                        pr = pair_tiles[k // 2]
                        off = (k % 2) * 128
                        # hi = fp8(relu(ps)); lo = fp8(relu(ps) - hi)
                        nc.scalar.activation(pr[:, off : off + H], ps[:], RELU)
                        nc.vector.scalar_tensor_tensor(
                            pr[:, off + H : off + 128],
                            ps[:],
                            0.0,
                            pr[:, off : off + H],
                            mybir.AluOpType.max,
                            mybir.AluOpType.subtract,
                        )

                # ------------- phase 2: two message-passing rounds ----------
                cur_pairs = [
                    pr[:].rearrange("p (s c) -> p s c", s=2) for pr in pair_tiles
                ]
                pair_order = list(range(NPAIR))
                for rnd in (1, 2):
                    with (
                        tc.tile_pool(name=f"rd{rnd}", bufs=1) as rd,
                        tc.tile_pool(name=f"prd{rnd}", bufs=1, space="PSUM") as prd,
                    ):
                        # aggT parts: psum rows 0:64 = (A'@hi)T, 64:128 =
                        # (A'@lo)T, then h_newT = relu(W_conv^T @ agg' + b)
                        # via bf16 parts of agg against bf16 hi/lo of W_conv.
                        for n in range(2):
                            nsl = slice(n * 512, (n + 1) * 512)
                            psa = prd.tile([128, 512], F32, tag="psa", bufs=2)
                            if n == 0:
                                absorb(psa, 128, 512)
                            for ji, j in enumerate(pair_order):
                                nc.tensor.matmul(
                                    psa[:],
                                    cur_pairs[j],
                                    a2_tiles[j][
                                        :, n * NB : (n + 1) * NB
                                    ].rearrange("p (t w) -> p t w", t=2),
                                    start=(ji == 0),
                                    stop=(ji == NPAIR - 1),
                                    perf_mode=DR,
                                )
                            agg_hi = rd.tile([128, 512], BF16, tag="agghi", bufs=2)
                            nc.vector.tensor_copy(agg_hi[:], psa[:])
                            agg_h32 = rd.tile([128, 512], F32, tag="aggh32", bufs=2)
                            nc.vector.tensor_copy(agg_h32[:], agg_hi[:])
                            agg_lo = rd.tile([128, 512], BF16, tag="agglo", bufs=2)
                            nc.vector.tensor_sub(agg_lo[:], psa[:], agg_h32[:])
                            psw = prd.tile([H, 512], F32, tag="psw", bufs=2)
                            if n == 0:
                                absorb(psw, H, 512)
                            nc.tensor.matmul(
                                psw[:], wc2h_s[:], agg_hi[:], start=True, stop=False
                            )
                            nc.tensor.matmul(
                                psw[:], wc2h_s[:], agg_lo[:], start=False, stop=False
                            )
                            nc.tensor.matmul(
                                psw[:], wc2l_s[:], agg_hi[:], start=False, stop=True
                            )
                            # fp8 2-component split of this 512-col half:
                            # hi = fp8(relu(psw+b)), lo = fp8(relu(psw+b)-hi)
                            if rnd == 2:
                                hi_h = hi8T[:, nsl]
                                lo_h = lo8T[:, nsl]
                            else:
                                hi_t = rd.tile([H, 512], F8, tag="hi8h", bufs=2)
                                lo_t = rd.tile([H, 512], F8, tag="lo8h", bufs=2)
                                hi_h, lo_h = hi_t[:], lo_t[:]
                            nc.scalar.activation(hi_h, psw[:], RELU, bias=bc_s[:])
                            h32 = rd.tile([H, 512], F32, tag="h32", bufs=2)
                            nc.vector.scalar_tensor_tensor(
                                h32[:],
                                psw[:],
                                bc_s[:],
                                zero64[:],
                                mybir.AluOpType.add,
                                mybir.AluOpType.max,
                            )
                            nc.vector.tensor_sub(lo_h, h32[:], hi_h)

                            if rnd == 1:
                                # transpose to node-major, stage + all-gather
                                # this half while the other half computes.
                                hiq = rd.tile([H, 512], BF16, tag="hiq", bufs=2)
                                nc.scalar.activation(hiq[:], hi_h, COPY)
                                loq = rd.tile([H, 512], BF16, tag="loq", bufs=2)
                                nc.scalar.activation(loq[:], lo_h, COPY)
                                agi, ago = (
                                    (ag1a_in, ag1a_out)
                                    if n == 0
                                    else (ag1b_in, ag1b_out)
                                )
                                for mm in range(MT // 2):
                                    pst = prd.tile([128, 128], BF16, tag="pst", bufs=2)
                                    nc.tensor.transpose(
                                        pst[:, 0:H],
                                        hiq[:, mm * 128 : (mm + 1) * 128],
                                        ident[0:H, 0:H],
                                    )
                                    nc.tensor.transpose(
                                        pst[:, H:128],
                                        loq[:, mm * 128 : (mm + 1) * 128],
                                        ident[0:H, 0:H],
                                    )
                                    nrm8 = rd.tile([128, 128], F8, tag="nrm", bufs=4)
                                    nc.vector.tensor_copy(nrm8[:], pst[:])
                                    nc.sync.dma_start(
                                        out=agi[mm * 128 : (mm + 1) * 128, :],
                                        in_=nrm8[:],
                                    )
                                nc.gpsimd.collective_compute(
                                    "AllGather",
                                    mybir.AluOpType.bypass,
                                    replica_groups=rg,
                                    ins=[agi[:]],
                                    outs=[ago[:]],
                                )
                            else:
                                # stage H-major fp8 comps for the phase-3 rhs
                                nc.sync.dma_start(
                                    out=ag2_in[0:H, nsl], in_=hi_h
                                )
                                nc.sync.dma_start(
                                    out=ag2_in[H:128, nsl], in_=lo_h
                                )

                        if rnd == 1:
                            # unpack gathered fp8 comps into round-2 lhsT
                            # pair views. gathered rows r of half hf map to
                            # global k-tile (r//512)*8 + hf*4 + (r%512)//128.
                            cur_pairs = [None] * NPAIR
                            order = []
                            for hf, ago in [(0, ag1a_out), (1, ag1b_out)]:
                                for g in range(8):
                                    hl8 = hilopool.tile(
                                        [128, 512], F8,
                                        name=f"h1hl{hf}_{g}", tag="HL8", bufs=16,
                                    )
                                    nc.sync.dma_start(
                                        out=hl8[:].rearrange(
                                            "p (t c) -> p t c", t=4
                                        ),
                                        in_=ago[
                                            g * 512 : (g + 1) * 512, :
                                        ].rearrange("(t p) c -> p t c", p=128),
                                    )
                                    for pp in range(2):
                                        j = (8 * g + 4 * hf) // 2 + pp
                                        cur_pairs[j] = hl8[
                                            :, pp * 256 : (pp + 1) * 256
                                        ].rearrange("p (s c) -> p s c", s=2)
                                        order.append(j)
                            pair_order = order
                        else:
                            nc.gpsimd.collective_compute(
                                "AllGather",
                                mybir.AluOpType.bypass,
                                replica_groups=rg,
                                ins=[ag2_in[:]],
                                outs=[ag2_out[:]],
                            )

            # ---------------- phase 3: sim / fdeps + output -----------------
            # (A/hilo pools released -> plenty of SBUF for the fp8 operands)
            # sim tile = 2 DoubleRow matmuls; fdeps tile = sim psum * rowmask
            # (per-partition scalar, prescaled 0.25) * colmask: one DVE op.
            # Both staged as fp16 at 0.25 scale -> 32MB of output DMA.
            with (
                tc.tile_pool(name="ph3", bufs=1) as ph3,
                tc.tile_pool(name="stg", bufs=3) as stg,
                tc.tile_pool(name="pp3", bufs=8, space="PSUM") as pp3,
            ):
                # masks first: they don't depend on AG2, so they hide it
                mask_all = ph3.tile([1, N], BF16, tag="maskall")
                nc.sync.dma_start(out=mask_all[:], in_=nodes_all[:])
                nc.vector.tensor_single_scalar(
                    mask_all[:], mask_all[:], 2.0, mybir.AluOpType.is_equal
                )
                nodes_tp = ph3.tile([128, MT], F32, tag="nodestp")
                nc.sync.dma_start(out=nodes_tp[:], in_=nodes_ownT[:])
                maskT = ph3.tile([128, MT], F32, tag="maskT")
                nc.vector.tensor_single_scalar(
                    maskT[:], nodes_tp[:], 2.0, mybir.AluOpType.is_equal
                )
                # prescale row mask by the fp16 output scale
                nc.vector.tensor_single_scalar(
                    maskT[:], maskT[:], 0.25, mybir.AluOpType.mult
                )
                # column mask broadcast to 128 partitions (K=1 matmuls), f32
                colmask = ph3.tile([128, N], F32, tag="colmask")
                for n in range(NT):
                    nsl = slice(n * 512, (n + 1) * 512)
                    psm = pp3.tile([128, 512], F32, tag="ps3", bufs=8)
                    nc.tensor.matmul(
                        psm[:], ones_s[:], mask_all[:, nsl], start=True, stop=True
                    )
                    nc.vector.tensor_copy(colmask[:, nsl], psm[:])

                # stationary: cols (m, slot, 128): slot0 = [hi;lo], slot1 =
                # [hi;0] (partition-stacked along K=128)
                stat = ph3.tile([128, 2 * NB], F8, tag="stat")
                for m in range(MT):
                    msl = slice(m * 128, (m + 1) * 128)
                    base = m * 256
                    nc.vector.tensor_copy(stat[0:H, base : base + 128], hi8T[:, msl])
                    nc.vector.tensor_copy(
                        stat[H:128, base : base + 128], lo8T[:, msl]
                    )
                    nc.vector.tensor_copy(
                        stat[0:H, base + 128 : base + 256], hi8T[:, msl]
                    )
                    nc.vector.memset(stat[H:128, base + 128 : base + 256], 0.0)

                # moving operand: cols (n-tile, slot, 512): slot0 = [hi;hi],
                # slot1 = [lo;lo]; unpacked from the gathered fp8 comps.
                rhs_r = ph3.tile([128, 2 * N], F8, tag="rhs")
                for c in range(NCORES):
                    for nn in range(2):
                        base = (2 * c + nn) * 1024
                        csl = slice(nn * 512, (nn + 1) * 512)
                        hi_src = ag2_out[c * 128 : c * 128 + H, csl]
                        lo_src = ag2_out[c * 128 + H : (c + 1) * 128, csl]
                        nc.gpsimd.dma_start(
                            out=rhs_r[0:H, base : base + 512], in_=hi_src
                        )
                        nc.gpsimd.dma_start(
                            out=rhs_r[H:128, base : base + 512], in_=hi_src
                        )
                        nc.gpsimd.dma_start(
                            out=rhs_r[0:H, base + 512 : base + 1024], in_=lo_src
                        )
                        nc.gpsimd.dma_start(
                            out=rhs_r[H:128, base + 512 : base + 1024], in_=lo_src
                        )

                first = True
                for m in range(MT):
                    msl = slice(m * 128, (m + 1) * 128)
                    lhsT_m = stat[:, m * 256 : (m + 1) * 256].rearrange(
                        "p (s c) -> p s c", s=2
                    )
                    rowm = maskT[:, m : m + 1]
                    for ng in range(4):
                        ngsl = slice(ng * 2048, (ng + 1) * 2048)
                        stA = stg.tile([128, 2048], F16, tag="stA", bufs=3)
                        stB = stg.tile([128, 2048], F16, tag="stB", bufs=3)
                        for j in range(4):
                            n = ng * 4 + j
                            nsl = slice(n * 512, (n + 1) * 512)
                            jsl = slice(j * 512, (j + 1) * 512)
                            ps3 = pp3.tile([128, 512], F32, tag="ps3", bufs=8)
                            if first:
                                absorb(ps3, 128, 512)
                                first = False
                            nc.tensor.matmul(
                                ps3[:],
                                lhsT_m,
                                rhs_r[:, n * 1024 : (n + 1) * 1024].rearrange(
                                    "p (s w) -> p s w", s=2
                                ),
                                start=True,
                                stop=True,
                                perf_mode=DR,
                            )
                            nc.scalar.activation(
                                stA[:, jsl], ps3[:], COPY, scale=0.25
                            )
                            nc.vector.scalar_tensor_tensor(
                                stB[:, jsl],
                                ps3[:],
                                rowm,
                                colmask[:, nsl],
                                mybir.AluOpType.mult,
                                mybir.AluOpType.mult,
                            )
                        nc.sync.dma_start(out=out_ext[1, msl, ngsl], in_=stA[:])
                        nc.sync.dma_start(out=out_ext[0, msl, ngsl], in_=stB[:])
    _legalize_waits(nc)
    return nc


def _host_prep(features, W_node, b_node, W_conv, b_conv, nodes, edges):
    features = np.asarray(features, np.float32)
    W_node = np.asarray(W_node, np.float32)
    b_node = np.asarray(b_node, np.float32)
    W_conv = np.asarray(W_conv, np.float32)
    b_conv = np.asarray(b_conv, np.float32)
    nodes = np.asarray(nodes)
    edges = np.asarray(edges)

    def _hilo(x):
        hi = x.astype(ml_dtypes.bfloat16)
        lo = (x - hi.astype(np.float32)).astype(ml_dtypes.bfloat16)
        return hi, lo

    # [features.T; ones] and [W_node; b_node], K-stacked for bf16 hi/lo:
    # [fa_hi; fa_lo_z; fa_hi] . [Wa_hi; Wa_hi; Wa_lo] ~= f@W + b
    fa = np.concatenate([features.T, np.ones((1, N), np.float32)], axis=0)
    Wa = np.concatenate([W_node, b_node[None, :]], axis=0)
    fa_hi, fa_lo = _hilo(fa)
    fa_lo_z = fa_lo.copy()
    fa_lo_z[F, :] = 0  # no double-counted bias
    Wa_hi, Wa_lo = _hilo(Wa)
    featT3 = np.concatenate([fa_hi, fa_lo_z, fa_hi], axis=0)  # [33, N] bf16
    W3 = np.concatenate([Wa_hi, Wa_hi, Wa_lo], axis=0)  # [33, H] bf16

    Wc_hi, Wc_lo = _hilo(W_conv)
    Wc2h = np.concatenate([Wc_hi, Wc_hi], axis=0)  # [128, H] bf16
    Wc2l = np.concatenate([Wc_lo, Wc_lo], axis=0)
    bc = b_conv.reshape(H, 1)
    nodes_f = nodes.astype(np.float32).reshape(1, N)

    src = edges[:, 0].astype(np.int64)
    dst = edges[:, 1].astype(np.int64)
    in_maps = []
    for c in range(NCORES):
        sel = (dst >= c * NB) & (dst < (c + 1) * NB)
        idx = src[sel] * NB + (dst[sel] - c * NB)
        cnt = np.bincount(idx, minlength=N * NB).astype(np.float32).reshape(N, NB)
        cnt[c * NB + np.arange(NB), np.arange(NB)] += 1.0  # fold identity
        assert cnt.max() <= 16, "adjacency counts exceed exact fp8 range"
        in_maps.append(
            {
                "featT3": featT3,
                "W3": W3,
                "Wc2h": Wc2h,
                "Wc2l": Wc2l,
                "bc": bc,
                "nodes_ownT": np.ascontiguousarray(
                    nodes_f[0, c * NB : (c + 1) * NB].reshape(MT, 128).T
                ),
                "nodes_all": nodes_f.astype(ml_dtypes.bfloat16),
                "AT": cnt.astype(ml_dtypes.float8_e4m3),
            }
        )
    return in_maps


def kernel(features, W_node, b_node, W_conv, b_conv, nodes, edges, **kw):
    global LAST_RESULT
    _ensure_trace_hook()
    in_maps = _host_prep(features, W_node, b_node, W_conv, b_conv, nodes, edges)
    nc = _build_nc()
    res = run_bass_kernel_spmd(nc, in_maps, core_ids=list(range(NCORES)))
    LAST_RESULT = res
    out = np.empty((2, N, N), np.float32)
    for c in range(NCORES):
        blk = np.asarray(res.results[c]["out"], dtype=np.float32)
        out[:, c * NB : (c + 1) * NB, :] = blk * 4.0
    return out


if __name__ == "__main__":
    np.random.seed(0)
    feats = np.random.randn(N, F).astype(np.float32)
    ins = {
        "features": feats,
        "W_node": (np.random.randn(F, H) * 0.1).astype(np.float32),
        "b_node": (np.random.randn(H) * 0.1).astype(np.float32),
        "W_conv": (np.random.randn(H, H) * 0.05).astype(np.float32),
        "b_conv": (np.random.randn(H) * 0.05).astype(np.float32),
        "nodes": np.random.randint(0, 5, N, dtype=np.int32),
        "edges": np.random.randint(0, N, (524288, 2), dtype=np.int32),
    }
    out = kernel(**ins)
    print(out.shape, out.dtype)


# revision 9
# speedup vs baseline: 1.3330x; 1.0209x over previous
"""Trainium2 Bass kernel for the DependencyAnalyzer GNN problem.

Computation (reference semantics):
    h = relu(features @ W_node + b_node)                  # [N, H]
    2x: agg = scatter_add(h[src] -> dst);  h = relu((h + agg) @ W_conv + b_conv)
    out = stack([ (m*h) @ (m*h).T,  h @ h.T ])            # m = (nodes == 2)

Strategy (8 NeuronCores, SPMD):
  - Host reformats the edge list into per-core dense adjacency blocks
    A'^T [src=8192, dst_local=1024] in fp8e4 (counts <= 16 are exact),
    with the identity folded in (A' = A + S_c) so A' @ h == h + agg.
  - h is carried as a 2-component fp8e4 decomposition (hi + lo ~ 8 mantissa
    bits); the A' matmuls run in fp8 DoubleRow perf mode: each instruction
    consumes TWO k-tiles (lhsT [128,2,128] h-comps, rhs [128,2,512] A rows)
    at half the per-column cost of bf16.
  - Exactly two collectives (CC setup time scales with the count): one fp8
    AllGather per round exchanging the per-core h blocks (round 1 in
    node-major layout for the round-2 stationary, round 2 in H-major layout
    for the output-phase moving operand). The mask/colmask setup for the
    output phase is built while the round-1 AllGather is in flight.
  - similarity = (hi+lo)^T (hi+lo) via 2 DoubleRow matmuls per 512-col
    output tile (slots [hi;0],[lo;hi] against the natural [hi;lo] moving
    operand); the function_deps tile is the same psum times row/col masks
    (one DVE op, spread across vector and gpsimd).
  - Outputs are staged as float16 scaled by 0.25 (|out|/4 < 65504), DMA'd
    as 32MB/core instead of 64MB, and rescaled to fp32 on the host.
"""

import numpy as np
import ml_dtypes

import concourse.bass as bass
import concourse.mybir as mybir
import concourse.tile as tile
from concourse import masks
from concourse.bass_utils import run_bass_kernel_spmd

N = 8192          # nodes
NB = 1024         # nodes per core block
NCORES = 8
F = 10            # feature dim
FA = F + 1        # +1 ones row (bias fold)
H = 64            # hidden dim
KT = N // 128     # 64 src k-tiles
NPAIR = KT // 2   # 32 k-tile pairs (DoubleRow)
MT = NB // 128    # 8 own m-tiles
NT = N // 512     # 16 n-tiles of 512
F32 = mybir.dt.float32
BF16 = mybir.dt.bfloat16
F16 = mybir.dt.float16
F8 = mybir.dt.float8e4
RELU = mybir.ActivationFunctionType.Relu
COPY = mybir.ActivationFunctionType.Copy
DR = mybir.MatmulPerfMode.DoubleRow

LAST_RESULT = None  # BassKernelResults of the most recent run (for test harness)


def _ensure_trace_hook():
    """Best-effort: register the NTFF profiling hook for trace=True runs.

    The agent image's ``antenv`` package lacks ``axon_hooks``; recreate it
    in-process and install the ctypes-based hook from trn_agent_boot so
    ``run_bass_kernel_spmd(trace=True)`` can capture HW exec times.
    Silently no-ops if anything is missing — plain runs are unaffected.
    """
    import sys as _sys
    import types as _types

    try:
        if "antenv.axon_hooks" in _sys.modules:
            return
        import antenv as _antenv

        mod = _types.ModuleType("antenv.axon_hooks")
        _state = {"hook": None}
        mod.set_axon_ntff_profile_hook = lambda h: _state.__setitem__("hook", h)
        mod.get_axon_ntff_profile_hook = lambda: _state["hook"]
        _sys.modules["antenv.axon_hooks"] = mod
        _antenv.axon_hooks = mod

        from trn_agent_boot.trn_boot import _ntff_profile_via_ctypes

        so_path = "/opt/axon/libaxon_pjrt.so"
        import os as _os

        if _os.path.exists(so_path):
            hook = _ntff_profile_via_ctypes(so_path)
            if hook is not None:
                mod.set_axon_ntff_profile_hook(hook)
    except Exception:
        pass


def _legalize_waits(nc, max_waits=1):
    """This walrus build accepts at most one sync-wait per lowered HW
    instruction; hoist extra waits onto standalone EventSemaphore
    instructions on the same (in-order) engine queue."""
    n_fixed = 0
    for f in nc.m.functions:
        for bb in f.blocks:
            new_list = []
            for ins in bb.instructions:
                si = ins.sync_info
                if si is not None and len(si.on_wait) > max_waits:
                    waits = list(si.on_wait)
                    for w in waits[: len(waits) - max_waits]:
                        ev = mybir.InstEventSemaphore(
                            name=f"{ins.name}-w-{w.ant_name}",
                            ins=[],
                            outs=[],
                            sync_info=mybir.SyncInfo(on_wait=[w], on_update=[]),
                            engine=ins.engine,
                        )
                        new_list.append(ev)
                    ins.sync_info = mybir.SyncInfo(
                        on_wait=waits[len(waits) - max_waits :],
                        on_update=list(si.on_update),
                    )
                    n_fixed += 1
                new_list.append(ins)
            bb.instructions = new_list
    return n_fixed


def _build_nc():
    nc = bass.Bass(num_devices=NCORES)

    # ---- external I/O (same program on all cores; per-core data differs) ----
    # featT3/W3: K-stacked bf16 hi/lo decomposition of [features.T; ones] and
    # [W_node; b_node] so one bf16 matmul computes the fp32-accurate product:
    # [f_hi; f_lo; f_hi] . [W_hi; W_hi; W_lo] = f.W + b - f_lo.W_lo
    featT = nc.declare_dram_parameter("featT3", [3 * FA, N], BF16, isOutput=False)
    WnA = nc.declare_dram_parameter("W3", [3 * FA, H], BF16, isOutput=False)
    Wc2h = nc.declare_dram_parameter("Wc2h", [2 * H, H], BF16, isOutput=False)
    Wc2l = nc.declare_dram_parameter("Wc2l", [2 * H, H], BF16, isOutput=False)
    bc = nc.declare_dram_parameter("bc", [H, 1], F32, isOutput=False)
    nodes_ownT = nc.declare_dram_parameter("nodes_ownT", [128, MT], F32, isOutput=False)
    nodes_all = nc.declare_dram_parameter("nodes_all", [1, N], BF16, isOutput=False)
    AT = nc.declare_dram_parameter("AT", [N, NB], F8, isOutput=False)
    out_ext = nc.declare_dram_parameter("out", [2, NB, N], F16, isOutput=True)

    # ---- internal DRAM (collective bounce buffers) ----
    ag1_in = nc.dram_tensor("ag1_in", [NB, 128], F8)
    ag1_out = nc.dram_tensor("ag1_out", [N, 128], F8, addr_space="Shared")
    ag2_in = nc.dram_tensor("ag2_in", [128, NB], F8)
    ag2_out = nc.dram_tensor("ag2_out", [NCORES * 128, NB], F8, addr_space="Shared")
    rg = [list(range(NCORES))]

    with tile.TileContext(nc, num_cores=NCORES) as tc:
        with tc.tile_pool(name="persist", bufs=1) as persist:
            # ---------------- constants / small inputs (issued first) -------
            wn_s = persist.tile([3 * FA, H], BF16)
            nc.sync.dma_start(out=wn_s[:], in_=WnA[:])
            wc2h_s = persist.tile([2 * H, H], BF16)
            nc.sync.dma_start(out=wc2h_s[:], in_=Wc2h[:])
            wc2l_s = persist.tile([2 * H, H], BF16)
            nc.sync.dma_start(out=wc2l_s[:], in_=Wc2l[:])
            bc_s = persist.tile([H, 1], F32)
            nc.sync.dma_start(out=bc_s[:], in_=bc[:])
            mask_all = persist.tile([1, N], BF16)
            nc.sync.dma_start(out=mask_all[:], in_=nodes_all[:])
            nodes_tp = persist.tile([128, MT], F32)
            nc.sync.dma_start(out=nodes_tp[:], in_=nodes_ownT[:])
            ident = persist.tile([128, 128], BF16)
            masks.make_identity(nc, ident[:])
            ones_s = persist.tile([1, 128], BF16)
            nc.vector.memset(ones_s[:], 1.0)
            dummy_s = persist.tile([1, 512], BF16)
            nc.vector.memset(dummy_s[:], 0.0)
            zero64 = persist.tile([H, 512], F32)
            nc.vector.memset(zero64[:], 0.0)

            def absorb(pt, parts, free):
                # Dummy full-tile matmul: soaks up PSUM pool-boundary WAR
                # waits on PE so real matmuls stay within the ISA's sync
                # wait budget.
                nc.tensor.matmul(
                    pt[:, :],
                    dummy_s[0:1, 0:parts],
                    dummy_s[0:1, 0:free],
                    start=True,
                    stop=True,
                )

            # persistent phase-3 operands / masks
            hi8T = persist.tile([H, NB], F8)     # own final-h fp8 comps, T
            lo8T = persist.tile([H, NB], F8)
            stat = persist.tile([128, 2 * NB], F8)
            rhs_r = persist.tile([128, 2 * N], F8)
            colmask = persist.tile([128, N], F32)
            maskT = persist.tile([128, MT], F32)

            with (
                tc.tile_pool(name="apool", bufs=NPAIR) as apool,
                tc.tile_pool(name="hilo", bufs=NPAIR + 8) as hilopool,
            ):
                # ------------- phase 1: h0 for all nodes (replicated) -------
                pair_tiles = []   # round-1 lhsT pair tiles [128, 256] fp8
                with (
                    tc.tile_pool(name="ph1", bufs=2) as ph1,
                    tc.tile_pool(name="pp1", bufs=4, space="PSUM") as pp1,
                ):
                    # features first so h0 overlaps the big A-load
                    ft_halves = []
                    for half in range(2):
                        ft_h = ph1.tile([3 * FA, N // 2], BF16, tag=f"ft{half}", bufs=1)
                        nc.sync.dma_start(
                            out=ft_h[:],
                            in_=featT[:, half * (N // 2) : (half + 1) * (N // 2)],
                        )
                        ft_halves.append(ft_h)

                    # adjacency pair blocks, fp8 resident in SBUF for both
                    # rounds; col layout (k-slot, 1024) — DoubleRow rhs APs
                    # are rearrange-then-slice views.
                    a2_tiles = []
                    for j in range(NPAIR):
                        at = apool.tile([128, 2 * NB], F8, name=f"a{j}", tag="A")
                        nc.sync.dma_start(
                            out=at[:].rearrange("p (t n) -> p t n", t=2),
                            in_=AT[j * 256 : (j + 1) * 256, :].rearrange(
                                "(t p) n -> p t n", p=128
                            ),
                        )
                        a2_tiles.append(at)

                    for k in range(KT):
                        ft_s = ft_halves[k // (KT // 2)]
                        kk = k % (KT // 2)
                        ps = pp1.tile([128, H], F32, tag="p64", bufs=4)
                        if k == 0:
                            absorb(ps, 128, H)
                        nc.tensor.matmul(
                            ps[:],
                            ft_s[:, kk * 128 : (kk + 1) * 128],
                            wn_s[:],
                            start=True,
                            stop=True,
                        )
                        if k % 2 == 0:
                            pr = hilopool.tile(
                                [128, 256], F8, name=f"h0p{k // 2}", tag="HP"
                            )
                            pair_tiles.append(pr)
                        pr = pair_tiles[k // 2]
                        off = (k % 2) * 128
                        # hi = fp8(relu(ps)); lo = fp8(relu(ps) - hi)
                        nc.scalar.activation(pr[:, off : off + H], ps[:], RELU)
                        nc.vector.scalar_tensor_tensor(
                            pr[:, off + H : off + 128],
                            ps[:],
                            0.0,
                            pr[:, off : off + H],
                            mybir.AluOpType.max,
                            mybir.AluOpType.subtract,
                        )

                # ------------- phase 2: two message-passing rounds ----------
                cur_pairs = [
                    pr[:].rearrange("p (s c) -> p s c", s=2) for pr in pair_tiles
                ]
                for rnd in (1, 2):
                    with (
                        tc.tile_pool(name=f"rd{rnd}", bufs=1) as rd,
                        tc.tile_pool(name=f"prd{rnd}", bufs=1, space="PSUM") as prd,
                    ):
                        # aggT parts: psum rows 0:64 = (A'@hi)T, 64:128 =
                        # (A'@lo)T, then h_newT = relu(W_conv^T @ agg' + b)
                        # via bf16 parts of agg against bf16 hi/lo of W_conv.
                        for n in range(2):
                            nsl = slice(n * 512, (n + 1) * 512)
                            psa = prd.tile([128, 512], F32, tag="psa", bufs=2)
                            if n == 0:
                                absorb(psa, 128, 512)
                            for ji in range(NPAIR):
                                nc.tensor.matmul(
                                    psa[:],
                                    cur_pairs[ji],
                                    a2_tiles[ji][:].rearrange(
                                        "p (t w) -> p t w", t=2
                                    )[:, :, nsl],
                                    start=(ji == 0),
                                    stop=(ji == NPAIR - 1),
                                    perf_mode=DR,
                                )
                            agg_hi = rd.tile([128, 512], BF16, tag="agghi", bufs=2)
                            nc.vector.tensor_copy(agg_hi[:], psa[:])
                            agg_h32 = rd.tile([128, 512], F32, tag="aggh32", bufs=2)
                            nc.vector.tensor_copy(agg_h32[:], agg_hi[:])
                            agg_lo = rd.tile([128, 512], BF16, tag="agglo", bufs=2)
                            nc.vector.tensor_sub(agg_lo[:], psa[:], agg_h32[:])
                            psw = prd.tile([H, 512], F32, tag="psw", bufs=2)
                            if n == 0:
                                absorb(psw, H, 512)
                            nc.tensor.matmul(
                                psw[:], wc2h_s[:], agg_hi[:], start=True, stop=False
                            )
                            nc.tensor.matmul(
                                psw[:], wc2h_s[:], agg_lo[:], start=False, stop=False
                            )
                            nc.tensor.matmul(
                                psw[:], wc2l_s[:], agg_hi[:], start=False, stop=True
                            )
                            # fp8 2-component split of this 512-col half:
                            # hi = fp8(relu(psw+b)), lo = fp8(relu(psw+b)-hi)
                            if rnd == 2:
                                hi_h = hi8T[:, nsl]
                                lo_h = lo8T[:, nsl]
                            else:
                                hi_t = rd.tile([H, 512], F8, tag="hi8h", bufs=2)
                                lo_t = rd.tile([H, 512], F8, tag="lo8h", bufs=2)
                                hi_h, lo_h = hi_t[:], lo_t[:]
                            nc.scalar.activation(hi_h, psw[:], RELU, bias=bc_s[:])
                            h32 = rd.tile([H, 512], F32, tag="h32", bufs=2)
                            nc.vector.scalar_tensor_tensor(
                                h32[:],
                                psw[:],
                                bc_s[:],
                                zero64[:],
                                mybir.AluOpType.add,
                                mybir.AluOpType.max,
                            )
                            nc.vector.tensor_sub(lo_h, h32[:], hi_h)

                            if rnd == 1:
                                # transpose to node-major and stage for the
                                # single round-1 all-gather.
                                hiq = rd.tile([H, 512], BF16, tag="hiq", bufs=2)
                                nc.scalar.activation(hiq[:], hi_h, COPY)
                                loq = rd.tile([H, 512], BF16, tag="loq", bufs=2)
                                nc.scalar.activation(loq[:], lo_h, COPY)
                                for mm in range(MT // 2):
                                    m = n * (MT // 2) + mm
                                    pst = prd.tile([128, 128], BF16, tag="pst", bufs=2)
                                    nc.tensor.transpose(
                                        pst[:, 0:H],
                                        hiq[:, mm * 128 : (mm + 1) * 128],
                                        ident[0:H, 0:H],
                                    )
                                    nc.tensor.transpose(
                                        pst[:, H:128],
                                        loq[:, mm * 128 : (mm + 1) * 128],
                                        ident[0:H, 0:H],
                                    )
                                    nrm8 = rd.tile([128, 128], F8, tag="nrm", bufs=4)
                                    nc.vector.tensor_copy(nrm8[:], pst[:])
                                    nc.sync.dma_start(
                                        out=ag1_in[m * 128 : (m + 1) * 128, :],
                                        in_=nrm8[:],
                                    )
                            else:
                                # stage H-major fp8 comps for the phase-3 rhs
                                nc.sync.dma_start(out=ag2_in[0:H, nsl], in_=hi_h)
                                nc.sync.dma_start(out=ag2_in[H:128, nsl], in_=lo_h)

                        if rnd == 1:
                            nc.gpsimd.collective_compute(
                                "AllGather",
                                mybir.AluOpType.bypass,
                                replica_groups=rg,
                                ins=[ag1_in[:]],
                                outs=[ag1_out[:]],
                            )
                            # masks for phase 3: independent of the exchange,
                            # built while the all-gather is in flight.
                            nc.vector.tensor_single_scalar(
                                mask_all[:], mask_all[:], 2.0,
                                mybir.AluOpType.is_equal,
                            )
                            nc.vector.tensor_single_scalar(
                                maskT[:], nodes_tp[:], 2.0, mybir.AluOpType.is_equal
                            )
                            nc.vector.tensor_single_scalar(
                                maskT[:], maskT[:], 0.25, mybir.AluOpType.mult
                            )
                            for nn in range(NT):
                                nsl2 = slice(nn * 512, (nn + 1) * 512)
                                psm = prd.tile([128, 512], F32, tag="psm", bufs=2)
                                if nn == 0:
                                    absorb(psm, 128, 512)
                                nc.tensor.matmul(
                                    psm[:], ones_s[:], mask_all[:, nsl2],
                                    start=True, stop=True,
                                )
                                nc.vector.tensor_copy(colmask[:, nsl2], psm[:])
                            # unpack gathered node-major fp8 comps into
                            # round-2 lhsT pair views (4 pairs per DMA).
                            cur_pairs = []
                            for g in range(8):
                                hl8 = hilopool.tile(
                                    [128, 1024], F8, name=f"h1hl{g}", tag="HL8",
                                    bufs=8,
                                )
                                nc.sync.dma_start(
                                    out=hl8[:].rearrange("p (t c) -> p t c", t=8),
                                    in_=ag1_out[
                                        g * 1024 : (g + 1) * 1024, :
                                    ].rearrange("(t p) c -> p t c", p=128),
                                )
                                for pp in range(4):
                                    cur_pairs.append(
                                        hl8[
                                            :, pp * 256 : (pp + 1) * 256
                                        ].rearrange("p (s c) -> p s c", s=2)
                                    )
                        else:
                            nc.gpsimd.collective_compute(
                                "AllGather",
                                mybir.AluOpType.bypass,
                                replica_groups=rg,
                                ins=[ag2_in[:]],
                                outs=[ag2_out[:]],
                            )

            # ---------------- phase 3: sim / fdeps + output -----------------
            # sim tile = 1 DoubleRow matmul (slots [hi;0],[lo;hi] x [hi;lo]);
            # fdeps tile = sim psum * rowmask (prescaled 0.25) * colmask, one
            # DVE op spread across vector/gpsimd. Outputs staged fp16 @ 0.25.
            with (
                tc.tile_pool(name="ph3", bufs=1) as ph3,
                tc.tile_pool(name="stg", bufs=3) as stg,
                tc.tile_pool(name="pp3", bufs=8, space="PSUM") as pp3,
            ):
                # stationary: cols (m, slot, 128): slot0 = [hi;0], slot1 =
                # [lo;hi] (partition-stacked along K=128)
                for m in range(MT):
                    msl = slice(m * 128, (m + 1) * 128)
                    base = m * 256
                    nc.vector.tensor_copy(stat[0:H, base : base + 128], hi8T[:, msl])
                    nc.vector.memset(stat[H:128, base : base + 128], 0.0)
                    nc.vector.tensor_copy(
                        stat[0:H, base + 128 : base + 256], lo8T[:, msl]
                    )
                    nc.vector.tensor_copy(
                        stat[H:128, base + 128 : base + 256], hi8T[:, msl]
                    )

                # moving operand: slot-major cols (slot, node): both slots
                # are the natural [hi; lo] gathered layout (2 DMAs of 1MB).
                for s in range(2):
                    nc.gpsimd.dma_start(
                        out=rhs_r[:, s * N : (s + 1) * N].rearrange(
                            "p (c w) -> p c w", c=NCORES
                        ),
                        in_=ag2_out[:].rearrange("(c p) w -> p c w", p=128),
                    )
                rhs_v = rhs_r[:].rearrange("p (s w) -> p s w", s=2)

                first = True
                for m in range(MT):
                    msl = slice(m * 128, (m + 1) * 128)
                    lhsT_m = stat[:, m * 256 : (m + 1) * 256].rearrange(
                        "p (s c) -> p s c", s=2
                    )
                    rowm = maskT[:, m : m + 1]
                    for ng in range(4):
                        ngsl = slice(ng * 2048, (ng + 1) * 2048)
                        stA = stg.tile([128, 2048], F16, tag="stA", bufs=3)
                        stB = stg.tile([128, 2048], F16, tag="stB", bufs=3)
                        for j in range(4):
                            n = ng * 4 + j
                            nsl = slice(n * 512, (n + 1) * 512)
                            jsl = slice(j * 512, (j + 1) * 512)
                            ps3 = pp3.tile([128, 512], F32, tag="ps3", bufs=8)
                            if first:
                                absorb(ps3, 128, 512)
                                first = False
                            nc.tensor.matmul(
                                ps3[:],
                                lhsT_m,
                                rhs_v[:, :, nsl],
                                start=True,
                                stop=True,
                                perf_mode=DR,
                            )
                            nc.scalar.activation(
                                stA[:, jsl], ps3[:], COPY, scale=0.25
                            )
                            nc.vector.scalar_tensor_tensor(
                                stB[:, jsl],
                                ps3[:],
                                rowm,
                                colmask[:, nsl],
                                mybir.AluOpType.mult,
                                mybir.AluOpType.mult,
                            )
                        nc.sync.dma_start(out=out_ext[1, msl, ngsl], in_=stA[:])
                        nc.sync.dma_start(out=out_ext[0, msl, ngsl], in_=stB[:])
    _legalize_waits(nc)
    return nc


def _host_prep(features, W_node, b_node, W_conv, b_conv, nodes, edges):
    features = np.asarray(features, np.float32)
    W_node = np.asarray(W_node, np.float32)
    b_node = np.asarray(b_node, np.float32)
    W_conv = np.asarray(W_conv, np.float32)
    b_conv = np.asarray(b_conv, np.float32)
    nodes = np.asarray(nodes)
    edges = np.asarray(edges)

    def _hilo(x):
        hi = x.astype(ml_dtypes.bfloat16)
        lo = (x - hi.astype(np.float32)).astype(ml_dtypes.bfloat16)
        return hi, lo

    # [features.T; ones] and [W_node; b_node], K-stacked for bf16 hi/lo:
    # [fa_hi; fa_lo_z; fa_hi] . [Wa_hi; Wa_hi; Wa_lo] ~= f@W + b
    fa = np.concatenate([features.T, np.ones((1, N), np.float32)], axis=0)
    Wa = np.concatenate([W_node, b_node[None, :]], axis=0)
    fa_hi, fa_lo = _hilo(fa)
    fa_lo_z = fa_lo.copy()
    fa_lo_z[F, :] = 0  # no double-counted bias
    Wa_hi, Wa_lo = _hilo(Wa)
    featT3 = np.concatenate([fa_hi, fa_lo_z, fa_hi], axis=0)  # [33, N] bf16
    W3 = np.concatenate([Wa_hi, Wa_hi, Wa_lo], axis=0)  # [33, H] bf16

    Wc_hi, Wc_lo = _hilo(W_conv)
    Wc2h = np.concatenate([Wc_hi, Wc_hi], axis=0)  # [128, H] bf16
    Wc2l = np.concatenate([Wc_lo, Wc_lo], axis=0)
    bc = b_conv.reshape(H, 1)
    nodes_f = nodes.astype(np.float32).reshape(1, N)

    src = edges[:, 0].astype(np.int64)
    dst = edges[:, 1].astype(np.int64)
    in_maps = []
    for c in range(NCORES):
        sel = (dst >= c * NB) & (dst < (c + 1) * NB)
        idx = src[sel] * NB + (dst[sel] - c * NB)
        cnt = np.bincount(idx, minlength=N * NB).astype(np.float32).reshape(N, NB)
        cnt[c * NB + np.arange(NB), np.arange(NB)] += 1.0  # fold identity
        assert cnt.max() <= 16, "adjacency counts exceed exact fp8 range"
        in_maps.append(
            {
                "featT3": featT3,
                "W3": W3,
                "Wc2h": Wc2h,
                "Wc2l": Wc2l,
                "bc": bc,
                "nodes_ownT": np.ascontiguousarray(
                    nodes_f[0, c * NB : (c + 1) * NB].reshape(MT, 128).T
                ),
                "nodes_all": nodes_f.astype(ml_dtypes.bfloat16),
                "AT": cnt.astype(ml_dtypes.float8_e4m3),
            }
        )
    return in_maps


def kernel(features, W_node, b_node, W_conv, b_conv, nodes, edges, **kw):
    global LAST_RESULT
    _ensure_trace_hook()
    in_maps = _host_prep(features, W_node, b_node, W_conv, b_conv, nodes, edges)
    nc = _build_nc()
    res = run_bass_kernel_spmd(nc, in_maps, core_ids=list(range(NCORES)))
    LAST_RESULT = res
    out = np.empty((2, N, N), np.float32)
    for c in range(NCORES):
        blk = np.asarray(res.results[c]["out"], dtype=np.float32)
        out[:, c * NB : (c + 1) * NB, :] = blk * 4.0
    return out


if __name__ == "__main__":
    np.random.seed(0)
    feats = np.random.randn(N, F).astype(np.float32)
    ins = {
        "features": feats,
        "W_node": (np.random.randn(F, H) * 0.1).astype(np.float32),
        "b_node": (np.random.randn(H) * 0.1).astype(np.float32),
        "W_conv": (np.random.randn(H, H) * 0.05).astype(np.float32),
        "b_conv": (np.random.randn(H) * 0.05).astype(np.float32),
        "nodes": np.random.randint(0, 5, N, dtype=np.int32),
        "edges": np.random.randint(0, N, (524288, 2), dtype=np.int32),
    }
    out = kernel(**ins)
    print(out.shape, out.dtype)


# revision 16
# speedup vs baseline: 1.3495x; 1.0124x over previous
"""Trainium2 Bass kernel for the DependencyAnalyzer GNN problem.

Computation (reference semantics):
    h = relu(features @ W_node + b_node)                  # [N, H]
    2x: agg = scatter_add(h[src] -> dst);  h = relu((h + agg) @ W_conv + b_conv)
    out = stack([ (m*h) @ (m*h).T,  h @ h.T ])            # m = (nodes == 2)

Strategy (8 NeuronCores, SPMD):
  - Host reformats the edge list into per-core dense adjacency blocks
    A'^T [src=8192, dst_local=1024] in fp8e4 (counts <= 16 are exact),
    with the identity folded in (A' = A + S_c) so A' @ h == h + agg.
  - h is carried as a 2-component fp8e4 decomposition (hi + lo ~ 8 mantissa
    bits); the A' matmuls run in fp8 DoubleRow perf mode: each instruction
    consumes TWO k-tiles (lhsT [128,2,128] h-comps, rhs [128,2,512] A rows)
    at half the per-column cost of bf16.
  - Exactly two collectives (CC setup time scales with the count): one fp8
    AllGather per round exchanging the per-core h blocks (round 1 in
    node-major layout for the round-2 stationary, round 2 in H-major layout
    for the output-phase moving operand). The mask/colmask setup for the
    output phase is built while the round-1 AllGather is in flight.
  - similarity = (hi+lo)^T (hi+lo) via 2 DoubleRow matmuls per 512-col
    output tile (slots [hi;0],[lo;hi] against the natural [hi;lo] moving
    operand); the function_deps tile is the same psum times row/col masks
    (one DVE op, spread across vector and gpsimd).
  - Outputs are staged as float16 scaled by 0.25 (|out|/4 < 65504), DMA'd
    as 32MB/core instead of 64MB, and rescaled to fp32 on the host.
"""

import numpy as np
import ml_dtypes

import concourse.bass as bass
import concourse.mybir as mybir
import concourse.tile as tile
from concourse import masks
from concourse.bass_utils import run_bass_kernel_spmd

N = 8192          # nodes
NB = 1024         # nodes per core block
NCORES = 8
F = 10            # feature dim
FA = F + 1        # +1 ones row (bias fold)
H = 64            # hidden dim
KT = N // 128     # 64 src k-tiles
NPAIR = KT // 2   # 32 k-tile pairs (DoubleRow)
MT = NB // 128    # 8 own m-tiles
NT = N // 512     # 16 n-tiles of 512
F32 = mybir.dt.float32
BF16 = mybir.dt.bfloat16
F16 = mybir.dt.float16
F8 = mybir.dt.float8e4
RELU = mybir.ActivationFunctionType.Relu
COPY = mybir.ActivationFunctionType.Copy
DR = mybir.MatmulPerfMode.DoubleRow

LAST_RESULT = None  # BassKernelResults of the most recent run (for test harness)


def _ensure_trace_hook():
    """Best-effort: register the NTFF profiling hook for trace=True runs.

    The agent image's ``antenv`` package lacks ``axon_hooks``; recreate it
    in-process and install the ctypes-based hook from trn_agent_boot so
    ``run_bass_kernel_spmd(trace=True)`` can capture HW exec times.
    Silently no-ops if anything is missing — plain runs are unaffected.
    """
    import sys as _sys
    import types as _types

    try:
        if "antenv.axon_hooks" in _sys.modules:
            return
        import antenv as _antenv

        mod = _types.ModuleType("antenv.axon_hooks")
        _state = {"hook": None}
        mod.set_axon_ntff_profile_hook = lambda h: _state.__setitem__("hook", h)
        mod.get_axon_ntff_profile_hook = lambda: _state["hook"]
        _sys.modules["antenv.axon_hooks"] = mod
        _antenv.axon_hooks = mod

        from trn_agent_boot.trn_boot import _ntff_profile_via_ctypes

        so_path = "/opt/axon/libaxon_pjrt.so"
        import os as _os

        if _os.path.exists(so_path):
            hook = _ntff_profile_via_ctypes(so_path)
            if hook is not None:
                mod.set_axon_ntff_profile_hook(hook)
    except Exception:
        pass


def _legalize_waits(nc, max_waits=1):
    """This walrus build accepts at most one sync-wait per lowered HW
    instruction; hoist extra waits onto standalone EventSemaphore
    instructions on the same (in-order) engine queue."""
    n_fixed = 0
    for f in nc.m.functions:
        for bb in f.blocks:
            new_list = []
            for ins in bb.instructions:
                si = ins.sync_info
                if si is not None and len(si.on_wait) > max_waits:
                    waits = list(si.on_wait)
                    for w in waits[: len(waits) - max_waits]:
                        ev = mybir.InstEventSemaphore(
                            name=f"{ins.name}-w-{w.ant_name}",
                            ins=[],
                            outs=[],
                            sync_info=mybir.SyncInfo(on_wait=[w], on_update=[]),
                            engine=ins.engine,
                        )
                        new_list.append(ev)
                    ins.sync_info = mybir.SyncInfo(
                        on_wait=waits[len(waits) - max_waits :],
                        on_update=list(si.on_update),
                    )
                    n_fixed += 1
                new_list.append(ins)
            bb.instructions = new_list
    return n_fixed


def _build_nc():
    nc = bass.Bass(num_devices=NCORES)

    # ---- external I/O (same program on all cores; per-core data differs) ----
    # featT3/W3: K-stacked bf16 hi/lo decomposition of [features.T; ones] and
    # [W_node; b_node] so one bf16 matmul computes the fp32-accurate product:
    # [f_hi; f_lo; f_hi] . [W_hi; W_hi; W_lo] = f.W + b - f_lo.W_lo
    featT = nc.declare_dram_parameter("featT3", [3 * FA, N], BF16, isOutput=False)
    WnA = nc.declare_dram_parameter("W3", [3 * FA, H], BF16, isOutput=False)
    Wc2h = nc.declare_dram_parameter("Wc2h", [2 * H, H], BF16, isOutput=False)
    Wc2l = nc.declare_dram_parameter("Wc2l", [2 * H, H], BF16, isOutput=False)
    bc = nc.declare_dram_parameter("bc", [H, 1], F32, isOutput=False)
    nodes_ownT = nc.declare_dram_parameter("nodes_ownT", [128, MT], F32, isOutput=False)
    nodes_all = nc.declare_dram_parameter("nodes_all", [1, N], BF16, isOutput=False)
    AT = nc.declare_dram_parameter("AT", [N, NB], F8, isOutput=False)
    out_ext = nc.declare_dram_parameter("out", [2, NB, N], F16, isOutput=True)

    # ---- internal DRAM (collective bounce buffers) ----
    ag1_in = nc.dram_tensor("ag1_in", [NB, 128], F8)
    ag1_out = nc.dram_tensor("ag1_out", [N, 128], F8, addr_space="Shared")
    ag2_in = nc.dram_tensor("ag2_in", [128, NB], F8)
    ag2_out = nc.dram_tensor("ag2_out", [NCORES * 128, NB], F8, addr_space="Shared")
    rg = [list(range(NCORES))]

    with tile.TileContext(nc, num_cores=NCORES) as tc:
        with tc.tile_pool(name="persist", bufs=1) as persist:
            # ---------------- constants / small inputs (issued first) -------
            wn_s = persist.tile([3 * FA, H], BF16)
            nc.sync.dma_start(out=wn_s[:], in_=WnA[:])
            wc2h_s = persist.tile([2 * H, H], BF16)
            nc.sync.dma_start(out=wc2h_s[:], in_=Wc2h[:])
            wc2l_s = persist.tile([2 * H, H], BF16)
            nc.sync.dma_start(out=wc2l_s[:], in_=Wc2l[:])
            bc_s = persist.tile([H, 1], F32)
            nc.sync.dma_start(out=bc_s[:], in_=bc[:])
            mask_all = persist.tile([1, N], BF16)
            nc.sync.dma_start(out=mask_all[:], in_=nodes_all[:])
            nodes_tp = persist.tile([128, MT], F32)
            nc.sync.dma_start(out=nodes_tp[:], in_=nodes_ownT[:])
            ident = persist.tile([128, 128], BF16)
            masks.make_identity(nc, ident[:])
            ones_s = persist.tile([1, 128], BF16)
            nc.vector.memset(ones_s[:], 1.0)
            dummy_s = persist.tile([1, 512], BF16)
            nc.vector.memset(dummy_s[:], 0.0)
            zero64 = persist.tile([H, 512], F32)
            nc.vector.memset(zero64[:], 0.0)

            def absorb(pt, parts, free):
                # Dummy full-tile matmul: soaks up PSUM pool-boundary WAR
                # waits on PE so real matmuls stay within the ISA's sync
                # wait budget.
                nc.tensor.matmul(
                    pt[:, :],
                    dummy_s[0:1, 0:parts],
                    dummy_s[0:1, 0:free],
                    start=True,
                    stop=True,
                )

            # persistent phase-3 operands / masks
            hi8T = persist.tile([H, NB], F8)     # own final-h fp8 comps, T
            lo8T = persist.tile([H, NB], F8)
            stat = persist.tile([128, 2 * NB], F8)
            rhs_r = persist.tile([128, 2 * N], F8)
            colmask = persist.tile([128, N], BF16)
            maskT = persist.tile([128, MT], F32)

            with (
                tc.tile_pool(name="apool", bufs=NPAIR) as apool,
                tc.tile_pool(name="hilo", bufs=NPAIR + 8) as hilopool,
            ):
                # ------------- phase 1: h0 for all nodes (replicated) -------
                pair_tiles = []   # round-1 lhsT pair tiles [128, 256] fp8
                with (
                    tc.tile_pool(name="ph1", bufs=2) as ph1,
                    tc.tile_pool(name="pp1", bufs=4, space="PSUM") as pp1,
                ):
                    # features on the gpsimd queue so the sync queue can
                    # stream the big A-load without issue-serialization
                    ft_halves = []
                    for half in range(2):
                        ft_h = ph1.tile([3 * FA, N // 2], BF16, tag=f"ft{half}", bufs=1)
                        nc.gpsimd.dma_start(
                            out=ft_h[:],
                            in_=featT[:, half * (N // 2) : (half + 1) * (N // 2)],
                        )
                        ft_halves.append(ft_h)

                    # adjacency pair blocks, fp8 resident in SBUF for both
                    # rounds; col layout (n-half, k-slot, 512) so the two
                    # DoubleRow slots are 512-adjacent (fast PE fetch), one
                    # 4-dim DMA per pair tile.
                    a2_tiles = []
                    for j in range(NPAIR):
                        at = apool.tile([128, 2 * NB], F8, name=f"a{j}", tag="A")
                        nc.sync.dma_start(
                            out=at[:].rearrange("p (h t n) -> p h t n", h=2, t=2),
                            in_=AT[j * 256 : (j + 1) * 256, :].rearrange(
                                "(t p) (h n) -> p h t n", p=128, n=512
                            ),
                        )
                        a2_tiles.append(at)

                    for k in range(0, KT, 2):
                        ft_s = ft_halves[k // (KT // 2)]
                        kk = k % (KT // 2)
                        # one psum bank holds the h0 of a k-tile PAIR
                        ps = pp1.tile([128, 128], F32, tag="p128", bufs=3)
                        if k == 0:
                            absorb(ps, 128, 128)
                        for t in range(2):
                            nc.tensor.matmul(
                                ps[:, t * H : (t + 1) * H],
                                ft_s[:, (kk + t) * 128 : (kk + t + 1) * 128],
                                wn_s[:],
                                start=True,
                                stop=True,
                            )
                        pr = hilopool.tile(
                            [128, 256], F8, name=f"h0p{k // 2}", tag="HP"
                        )
                        pair_tiles.append(pr)
                        prv = pr[:].rearrange("p (s c) -> p s c", s=2)
                        psv = ps[:].rearrange("p (s c) -> p s c", s=2)
                        # hi = fp8(relu(ps)); lo = fp8(relu(ps) - hi), both
                        # k-tiles of the pair in one strided op each
                        nc.scalar.activation(prv[:, :, 0:H], psv, RELU)
                        nc.vector.scalar_tensor_tensor(
                            prv[:, :, H:128],
                            psv,
                            0.0,
                            prv[:, :, 0:H],
                            mybir.AluOpType.max,
                            mybir.AluOpType.subtract,
                        )

                # ------------- phase 2: two message-passing rounds ----------
                cur_pairs = [
                    pr[:].rearrange("p (s c) -> p s c", s=2) for pr in pair_tiles
                ]
                for rnd in (1, 2):
                    with (
                        tc.tile_pool(name=f"rd{rnd}", bufs=1) as rd,
                        tc.tile_pool(name=f"prd{rnd}", bufs=1, space="PSUM") as prd,
                    ):
                        # aggT parts: psum rows 0:64 = (A'@hi)T, 64:128 =
                        # (A'@lo)T, then h_newT = relu(W_conv^T @ agg' + b)
                        # via bf16 parts of agg against bf16 hi/lo of W_conv.
                        for n in range(2):
                            nsl = slice(n * 512, (n + 1) * 512)
                            psa = prd.tile([128, 512], F32, tag="psa", bufs=2)
                            if n == 0:
                                absorb(psa, 128, 512)
                            for ji in range(NPAIR):
                                nc.tensor.matmul(
                                    psa[:],
                                    cur_pairs[ji],
                                    a2_tiles[ji][
                                        :, n * NB : (n + 1) * NB
                                    ].rearrange("p (t w) -> p t w", t=2),
                                    start=(ji == 0),
                                    stop=(ji == NPAIR - 1),
                                    perf_mode=DR,
                                )
                            agg_hi = rd.tile([128, 512], BF16, tag="agghi", bufs=2)
                            nc.vector.tensor_copy(agg_hi[:], psa[:])
                            agg_h32 = rd.tile([128, 512], F32, tag="aggh32", bufs=2)
                            nc.vector.tensor_copy(agg_h32[:], agg_hi[:])
                            agg_lo = rd.tile([128, 512], BF16, tag="agglo", bufs=2)
                            nc.vector.tensor_sub(agg_lo[:], psa[:], agg_h32[:])
                            psw = prd.tile([H, 512], F32, tag="psw", bufs=2)
                            if n == 0:
                                absorb(psw, H, 512)
                            nc.tensor.matmul(
                                psw[:], wc2h_s[:], agg_hi[:], start=True, stop=False
                            )
                            nc.tensor.matmul(
                                psw[:], wc2h_s[:], agg_lo[:], start=False, stop=False
                            )
                            nc.tensor.matmul(
                                psw[:], wc2l_s[:], agg_hi[:], start=False, stop=True
                            )
                            # fp8 2-component split of this 512-col half:
                            # hi = fp8(relu(psw+b)), lo = fp8(relu(psw+b)-hi)
                            if rnd == 2:
                                hi_h = hi8T[:, nsl]
                                lo_h = lo8T[:, nsl]
                            else:
                                hi_t = rd.tile([H, 512], F8, tag="hi8h", bufs=2)
                                lo_t = rd.tile([H, 512], F8, tag="lo8h", bufs=2)
                                hi_h, lo_h = hi_t[:], lo_t[:]
                            nc.scalar.activation(hi_h, psw[:], RELU, bias=bc_s[:])
                            h32 = rd.tile([H, 512], F32, tag="h32", bufs=2)
                            nc.vector.scalar_tensor_tensor(
                                h32[:],
                                psw[:],
                                bc_s[:],
                                zero64[:],
                                mybir.AluOpType.add,
                                mybir.AluOpType.max,
                            )
                            nc.vector.tensor_sub(lo_h, h32[:], hi_h)

                            if rnd == 1:
                                # transpose to node-major and stage for the
                                # single round-1 all-gather.
                                hiq = rd.tile([H, 512], BF16, tag="hiq", bufs=2)
                                nc.scalar.activation(hiq[:], hi_h, COPY)
                                loq = rd.tile([H, 512], BF16, tag="loq", bufs=2)
                                nc.scalar.activation(loq[:], lo_h, COPY)
                                for mm in range(MT // 2):
                                    m = n * (MT // 2) + mm
                                    pst = prd.tile([128, 128], BF16, tag="pst", bufs=2)
                                    nc.tensor.transpose(
                                        pst[:, 0:H],
                                        hiq[:, mm * 128 : (mm + 1) * 128],
                                        ident[0:H, 0:H],
                                    )
                                    nc.tensor.transpose(
                                        pst[:, H:128],
                                        loq[:, mm * 128 : (mm + 1) * 128],
                                        ident[0:H, 0:H],
                                    )
                                    nrm8 = rd.tile([128, 128], F8, tag="nrm", bufs=4)
                                    nc.vector.tensor_copy(nrm8[:], pst[:])
                                    nc.sync.dma_start(
                                        out=ag1_in[m * 128 : (m + 1) * 128, :],
                                        in_=nrm8[:],
                                    )
                            else:
                                # stage H-major fp8 comps for the phase-3 rhs
                                nc.sync.dma_start(out=ag2_in[0:H, nsl], in_=hi_h)
                                nc.sync.dma_start(out=ag2_in[H:128, nsl], in_=lo_h)

                        if rnd == 1:
                            nc.gpsimd.collective_compute(
                                "AllGather",
                                mybir.AluOpType.bypass,
                                replica_groups=rg,
                                ins=[ag1_in[:]],
                                outs=[ag1_out[:]],
                            )
                            # masks for phase 3: independent of the exchange,
                            # built while the all-gather is in flight.
                            nc.vector.tensor_single_scalar(
                                mask_all[:], mask_all[:], 2.0,
                                mybir.AluOpType.is_equal,
                            )
                            nc.vector.tensor_single_scalar(
                                maskT[:], nodes_tp[:], 2.0, mybir.AluOpType.is_equal
                            )
                            for nn in range(NT):
                                nsl2 = slice(nn * 512, (nn + 1) * 512)
                                psm = prd.tile([128, 512], F32, tag="psm", bufs=2)
                                if nn == 0:
                                    absorb(psm, 128, 512)
                                nc.tensor.matmul(
                                    psm[:], ones_s[:], mask_all[:, nsl2],
                                    start=True, stop=True,
                                )
                                nc.vector.tensor_copy(colmask[:, nsl2], psm[:])
                            # unpack gathered node-major fp8 comps into
                            # round-2 lhsT pair views (4 pairs per DMA).
                            cur_pairs = []
                            for g in range(8):
                                hl8 = hilopool.tile(
                                    [128, 1024], F8, name=f"h1hl{g}", tag="HL8",
                                    bufs=8,
                                )
                                nc.sync.dma_start(
                                    out=hl8[:].rearrange("p (t c) -> p t c", t=8),
                                    in_=ag1_out[
                                        g * 1024 : (g + 1) * 1024, :
                                    ].rearrange("(t p) c -> p t c", p=128),
                                )
                                for pp in range(4):
                                    cur_pairs.append(
                                        hl8[
                                            :, pp * 256 : (pp + 1) * 256
                                        ].rearrange("p (s c) -> p s c", s=2)
                                    )
                        else:
                            nc.gpsimd.collective_compute(
                                "AllGather",
                                mybir.AluOpType.bypass,
                                replica_groups=rg,
                                ins=[ag2_in[:]],
                                outs=[ag2_out[:]],
                            )

            # ---------------- phase 3: sim / fdeps + output -----------------
            # sim tile = 1 DoubleRow matmul (slots [hi;0],[lo;hi] x [hi;lo]);
            # fdeps tile = sim psum * rowmask (prescaled 0.25) * colmask, one
            # DVE op spread across vector/gpsimd. Outputs staged fp16 @ 0.25.
            with (
                tc.tile_pool(name="ph3", bufs=1) as ph3,
                tc.tile_pool(name="stg", bufs=3) as stg,
                tc.tile_pool(name="pp3", bufs=8, space="PSUM") as pp3,
            ):
                # stationary: cols (m, slot, 128): slot0 = [hi;0], slot1 =
                # [lo;hi] (partition-stacked along K=128)
                for m in range(MT):
                    msl = slice(m * 128, (m + 1) * 128)
                    base = m * 256
                    nc.vector.tensor_copy(stat[0:H, base : base + 128], hi8T[:, msl])
                    nc.vector.memset(stat[H:128, base : base + 128], 0.0)
                    nc.vector.tensor_copy(
                        stat[0:H, base + 128 : base + 256], lo8T[:, msl]
                    )
                    nc.vector.tensor_copy(
                        stat[H:128, base + 128 : base + 256], hi8T[:, msl]
                    )

                # moving operand: cols (n-tile, slot, 512) so the two slots
                # of each DoubleRow rhs view are 512-adjacent; both slots are
                # the natural [hi; lo] gathered layout (2 DMAs of 1MB).
                for s in range(2):
                    for a in range(2):
                        off = a * 1024 + s * 512
                        nc.gpsimd.dma_start(
                            out=rhs_r[:].rearrange("p (c r) -> p c r", r=2048)[
                                :, :, off : off + 512
                            ],
                            in_=ag2_out[:].rearrange("(c p) w -> p c w", p=128)[
                                :, :, a * 512 : (a + 1) * 512
                            ],
                        )

                first = True
                for m in range(MT):
                    msl = slice(m * 128, (m + 1) * 128)
                    lhsT_m = stat[:, m * 256 : (m + 1) * 256].rearrange(
                        "p (s c) -> p s c", s=2
                    )
                    rowm = maskT[:, m : m + 1]
                    for ng in range(4):
                        ngsl = slice(ng * 2048, (ng + 1) * 2048)
                        stA = stg.tile([128, 2048], F16, tag="stA", bufs=3)
                        stB = stg.tile([128, 2048], F16, tag="stB", bufs=3)
                        for j in range(4):
                            n = ng * 4 + j
                            nsl = slice(n * 512, (n + 1) * 512)
                            jsl = slice(j * 512, (j + 1) * 512)
                            ps3 = pp3.tile([128, 512], F32, tag="ps3", bufs=8)
                            if first:
                                absorb(ps3, 128, 512)
                                first = False
                            nc.tensor.matmul(
                                ps3[:],
                                lhsT_m,
                                rhs_r[:, n * 1024 : (n + 1) * 1024].rearrange(
                                    "p (s w) -> p s w", s=2
                                ),
                                start=True,
                                stop=True,
                                perf_mode=DR,
                            )
                            # stA = 0.25*sim in fp16 (scalar, one tile/group
                            # on vector to balance); stB masks the SBUF fp16
                            # copy (all-16-bit operands -> DVE 2x mode), with
                            # {0,1} masks this is bit-identical to masking
                            # the psum directly.
                            if j == 3:
                                nc.vector.tensor_single_scalar(
                                    stA[:, jsl], ps3[:], 0.25,
                                    mybir.AluOpType.mult,
                                )
                            else:
                                nc.scalar.activation(
                                    stA[:, jsl], ps3[:], COPY, scale=0.25
                                )
                            nc.vector.scalar_tensor_tensor(
                                stB[:, jsl],
                                stA[:, jsl],
                                rowm,
                                colmask[:, nsl],
                                mybir.AluOpType.mult,
                                mybir.AluOpType.mult,
                            )
                        nc.sync.dma_start(out=out_ext[1, msl, ngsl], in_=stA[:])
                        nc.sync.dma_start(out=out_ext[0, msl, ngsl], in_=stB[:])
    _legalize_waits(nc)
    return nc


def _host_prep(features, W_node, b_node, W_conv, b_conv, nodes, edges):
    features = np.asarray(features, np.float32)
    W_node = np.asarray(W_node, np.float32)
    b_node = np.asarray(b_node, np.float32)
    W_conv = np.asarray(W_conv, np.float32)
    b_conv = np.asarray(b_conv, np.float32)
    nodes = np.asarray(nodes)
    edges = np.asarray(edges)

    def _hilo(x):
        hi = x.astype(ml_dtypes.bfloat16)
        lo = (x - hi.astype(np.float32)).astype(ml_dtypes.bfloat16)
        return hi, lo

    # [features.T; ones] and [W_node; b_node], K-stacked for bf16 hi/lo:
    # [fa_hi; fa_lo_z; fa_hi] . [Wa_hi; Wa_hi; Wa_lo] ~= f@W + b
    fa = np.concatenate([features.T, np.ones((1, N), np.float32)], axis=0)
    Wa = np.concatenate([W_node, b_node[None, :]], axis=0)
    fa_hi, fa_lo = _hilo(fa)
    fa_lo_z = fa_lo.copy()
    fa_lo_z[F, :] = 0  # no double-counted bias
    Wa_hi, Wa_lo = _hilo(Wa)
    featT3 = np.concatenate([fa_hi, fa_lo_z, fa_hi], axis=0)  # [33, N] bf16
    W3 = np.concatenate([Wa_hi, Wa_hi, Wa_lo], axis=0)  # [33, H] bf16

    Wc_hi, Wc_lo = _hilo(W_conv)
    Wc2h = np.concatenate([Wc_hi, Wc_hi], axis=0)  # [128, H] bf16
    Wc2l = np.concatenate([Wc_lo, Wc_lo], axis=0)
    bc = b_conv.reshape(H, 1)
    nodes_f = nodes.astype(np.float32).reshape(1, N)

    src = edges[:, 0].astype(np.int64)
    dst = edges[:, 1].astype(np.int64)
    in_maps = []
    for c in range(NCORES):
        sel = (dst >= c * NB) & (dst < (c + 1) * NB)
        idx = src[sel] * NB + (dst[sel] - c * NB)
        cnt = np.bincount(idx, minlength=N * NB).astype(np.float32).reshape(N, NB)
        cnt[c * NB + np.arange(NB), np.arange(NB)] += 1.0  # fold identity
        assert cnt.max() <= 16, "adjacency counts exceed exact fp8 range"
        in_maps.append(
            {
                "featT3": featT3,
                "W3": W3,
                "Wc2h": Wc2h,
                "Wc2l": Wc2l,
                "bc": bc,
                "nodes_ownT": np.ascontiguousarray(
                    nodes_f[0, c * NB : (c + 1) * NB].reshape(MT, 128).T
                ),
                "nodes_all": nodes_f.astype(ml_dtypes.bfloat16),
                "AT": cnt.astype(ml_dtypes.float8_e4m3),
            }
        )
    return in_maps


def kernel(features, W_node, b_node, W_conv, b_conv, nodes, edges, **kw):
    global LAST_RESULT
    _ensure_trace_hook()
    in_maps = _host_prep(features, W_node, b_node, W_conv, b_conv, nodes, edges)
    nc = _build_nc()
    res = run_bass_kernel_spmd(nc, in_maps, core_ids=list(range(NCORES)))
    LAST_RESULT = res
    out = np.empty((2, N, N), np.float32)
    for c in range(NCORES):
        blk = np.asarray(res.results[c]["out"], dtype=np.float32)
        out[:, c * NB : (c + 1) * NB, :] = blk * 4.0
    return out


if __name__ == "__main__":
    np.random.seed(0)
    feats = np.random.randn(N, F).astype(np.float32)
    ins = {
        "features": feats,
        "W_node": (np.random.randn(F, H) * 0.1).astype(np.float32),
        "b_node": (np.random.randn(H) * 0.1).astype(np.float32),
        "W_conv": (np.random.randn(H, H) * 0.05).astype(np.float32),
        "b_conv": (np.random.randn(H) * 0.05).astype(np.float32),
        "nodes": np.random.randint(0, 5, N, dtype=np.int32),
        "edges": np.random.randint(0, N, (524288, 2), dtype=np.int32),
    }
    out = kernel(**ins)
    print(out.shape, out.dtype)
